# revision 5
# baseline (speedup 1.0000x reference)
"""3-layer GAT + global pool on 8 Trainium2 NeuronCores (Bass/Tile SPMD).

Strategy: shard edges by destination node across the 8 cores (2500 dst
nodes each). Per layer, each core computes the node table [al_d|al_s|xp]
for its node shard, an AllGather replicates the full table to DRAM on
every core, and all per-edge work (gather by src, attention softmax,
weighted scatter-add via one-hot matmuls) is core-local. The final
pooled [64, HID] partials are summed on the host, which also applies the
final BatchNorm + FC (trivially small).
"""
import numpy as np

# ---- model constants (must match the reference) ----
N = 20000
E = 320000
G = 64
H = 4
C = 64
HID = H * C          # 256
IN = 128
LAT = 64
NEG = 0.2
BN_EPS = 1e-5
NL = 3

NCORE = 8
NSH = N // NCORE     # 2500 dst nodes per core
BLK = 128
NBLK = (NSH + BLK - 1) // BLK   # 20 (last block 68 nodes)
FTAB = 320           # table row: [al_d(4) | al_s(4) | xp(256) | pad(56)]
ROWW = 8 + HID       # 264 useful cols

_cache = {}


def _wrap16(idx):
    """[..., NIDX] int -> [..., 128, NIDX//16] int16 wrapped layout:
    element i at [i%16, i//16], replicated across the 8 groups of 16."""
    lead = idx.shape[:-1]
    nidx = idx.shape[-1]
    t = idx.reshape(*lead, nidx // 16, 16)
    t = np.swapaxes(t, -1, -2)  # [..., 16, nidx//16]
    out = np.broadcast_to(t[..., None, :, :], (*lead, 8, 16, nidx // 16))
    return np.ascontiguousarray(out.reshape(*lead, 128, nidx // 16)).astype(np.int16)


def _preprocess(edge_index, batch):
    src = np.concatenate([np.asarray(edge_index[0]), np.arange(N)]).astype(np.int64)
    dst = np.concatenate([np.asarray(edge_index[1]), np.arange(N)]).astype(np.int64)
    order = np.argsort(dst, kind="stable")
    srcs = src[order]
    dsts = dst[order]

    # block boundaries: core c, block b covers dst in [c*NSH + b*BLK, ...+BLK)
    bounds = []
    for c in range(NCORE):
        for b in range(NBLK):
            bounds.append(c * NSH + b * BLK)
    bounds.append(N)
    cuts = np.searchsorted(dsts, np.array(bounds))
    cnts = np.diff(cuts)
    slots = int(np.ceil(cnts.max() / BLK))
    nidx_blk = slots * BLK

    srcpad = np.zeros((NCORE, NBLK, nidx_blk), dtype=np.int64)
    dstpad = np.zeros((NCORE, NBLK, nidx_blk), dtype=np.int64)
    dloc = np.full((NCORE, NBLK, nidx_blk), -1.0, dtype=np.float32)
    for c in range(NCORE):
        for b in range(NBLK):
            k = c * NBLK + b
            lo, hi = cuts[k], cuts[k + 1]
            n = hi - lo
            srcpad[c, b, :n] = srcs[lo:hi]
            dstpad[c, b, :n] = dsts[lo:hi]
            dloc[c, b, :n] = (dsts[lo:hi] - (c * NSH + b * BLK)).astype(np.float32)

    srcw = _wrap16(srcpad.reshape(NCORE, NBLK * nidx_blk))  # [NC,128,NBLK*nidx/16]
    dstw = _wrap16(dstpad.reshape(NCORE, NBLK * nidx_blk))
    # dloc -> [NC, 128, NBLK*SLOTS]; [p, b*SLOTS+s] = dloc[b, s*128+p]
    dl = dloc.reshape(NCORE, NBLK, slots, BLK).transpose(0, 3, 1, 2)
    dl = np.ascontiguousarray(dl.reshape(NCORE, BLK, NBLK * slots))

    bat = np.asarray(batch).astype(np.int64)
    poolv = np.full((NCORE, BLK, NBLK), -1.0, dtype=np.float32)
    for c in range(NCORE):
        for b in range(NBLK):
            lo = c * NSH + b * BLK
            n = min(BLK, c * NSH + NSH - lo)
            poolv[c, :n, b] = bat[lo:lo + n].astype(np.float32)
    return srcw, dstw, dl, poolv, slots


def _amat(att):  # att [H, C] -> [HID, H] block diagonal
    A = np.zeros((H, C, H), dtype=np.float32)
    for h in range(H):
        A[h, :, h] = att[h]
    return A.reshape(HID, H)


def _build(slots):
    import concourse.bacc as bacc
    import concourse.mybir as mybir
    import concourse.tile as tile

    f32 = mybir.dt.float32
    i16 = mybir.dt.int16
    nidx_blk = slots * BLK

    nc = bacc.Bacc("TRN2", target_bir_lowering=False, debug=False,
                   enable_asserts=True, num_devices=NCORE)

    # ---- inputs ----
    xT_d = nc.dram_tensor("xT", [IN, NSH], f32, kind="ExternalInput")
    w0_d = nc.dram_tensor("w0cat", [IN, ROWW], f32, kind="ExternalInput")
    wc_d = nc.dram_tensor("wcat", [128, (NL - 1) * 2 * ROWW], f32, kind="ExternalInput")
    sb_d = nc.dram_tensor("sb", [128, NL * HID], f32, kind="ExternalInput")
    shb_d = nc.dram_tensor("shb", [128, NL * HID], f32, kind="ExternalInput")
    iota_d = nc.dram_tensor("iota", [128, 128], f32, kind="ExternalInput")
    ident_d = nc.dram_tensor("ident", [128, 128], f32, kind="ExternalInput")
    srcw_d = nc.dram_tensor("srcw", [128, NBLK * (nidx_blk // 16)], i16, kind="ExternalInput")
    dstw_d = nc.dram_tensor("dstw", [128, NBLK * (nidx_blk // 16)], i16, kind="ExternalInput")
    dloc_d = nc.dram_tensor("dloc", [128, NBLK * slots], f32, kind="ExternalInput")
    poolv_d = nc.dram_tensor("poolv", [128, NBLK], f32, kind="ExternalInput")

    pool_d = nc.dram_tensor("pool_out", [G, HID], f32, kind="ExternalOutput")

    IW = nidx_blk // 16

    with tile.TileContext(nc) as tc:
        with (
            tc.tile_pool(name="const", bufs=1) as constp,
            tc.tile_pool(name="hbuf", bufs=1) as hbufp,
            tc.tile_pool(name="work", bufs=2) as workp,
            tc.tile_pool(name="slotw", bufs=4) as slotp,
            tc.tile_pool(name="small", bufs=3) as smallp,
            tc.tile_pool(name="psum", bufs=2, space="PSUM") as psump,
            tc.tile_pool(name="psacc", bufs=1, space="PSUM") as psaccp,
            tc.tile_pool(name="dram", bufs=1, space="DRAM") as dramp,
        ):
            # ---- load constants ----
            xT = constp.tile([IN, NSH], f32, tag="xT")
            nc.sync.dma_start(xT[:], xT_d[:])
            w0 = constp.tile([IN, ROWW], f32, tag="w0")
            nc.sync.dma_start(w0[:], w0_d[:])
            wc = constp.tile([128, (NL - 1) * 2 * ROWW], f32, tag="wc")
            nc.sync.dma_start(wc[:], wc_d[:])
            sb = constp.tile([128, NL * HID], f32, tag="sb")
            nc.sync.dma_start(sb[:], sb_d[:])
            shb = constp.tile([128, NL * HID], f32, tag="shb")
            nc.sync.dma_start(shb[:], shb_d[:])
            iota = constp.tile([128, 128], f32, tag="iota")
            nc.sync.dma_start(iota[:], iota_d[:])
            ident = constp.tile([128, 128], f32, tag="ident")
            nc.sync.dma_start(ident[:], ident_d[:])
            srcw = constp.tile([128, NBLK * IW], i16, tag="srcw")
            nc.sync.dma_start(srcw[:], srcw_d[:])
            dstw = constp.tile([128, NBLK * IW], i16, tag="dstw")
            nc.sync.dma_start(dstw[:], dstw_d[:])
            dloc = constp.tile([128, NBLK * slots], f32, tag="dloc")
            nc.sync.dma_start(dloc[:], dloc_d[:])
            poolv = constp.tile([128, NBLK], f32, tag="poolv")
            nc.sync.dma_start(poolv[:], poolv_d[:])

            # persistent hT across a layer
            hT0 = hbufp.tile([128, NSH], f32, tag="hT0")
            hT1 = hbufp.tile([128, NSH], f32, tag="hT1")

            pool_ps = psaccp.tile([G, HID], f32, tag="poolps")

            def blkw(b):  # width of dst-block b
                return min(BLK, NSH - b * BLK)

            # ---- layer-1 table: rows [al_d | al_s | xp1] from x ----
            agin = dramp.tile([NSH, FTAB], f32, tag="agin0")
            for b in range(NBLK):
                w = blkw(b)
                ps = psump.tile([128, ROWW], f32, tag="xp_ps")
                nc.tensor.matmul(ps[:w, :], xT[:, b * BLK:b * BLK + w], w0[:],
                                 start=True, stop=True)
                row = smallp.tile([128, ROWW], f32, tag="tabrow")
                nc.scalar.copy(row[:w, :], ps[:w, :])
                nc.sync.dma_start(agin[b * BLK:b * BLK + w, 0:ROWW], row[:w, :])

            for layer in range(NL):
                # ---- AllGather the node table ----
                t1 = dramp.tile([N, FTAB], f32, tag=f"t1_{layer}")
                nc.gpsimd.collective_compute(
                    "AllGather", mybir.AluOpType.bypass,
                    ins=[agin.opt()], outs=[t1.opt()],
                    replica_groups=[list(range(NCORE))],
                )
                if layer < NL - 1:
                    agin = dramp.tile([NSH, FTAB], f32, tag=f"agin{layer + 1}")

                for b in range(NBLK):
                    w = blkw(b)
                    # ---- gathers ----
                    g1 = workp.tile([128, slots, FTAB], f32, tag="g1")
                    nc.gpsimd.dma_gather(
                        g1[:], t1[:], srcw[:, b * IW:(b + 1) * IW],
                        num_idxs=nidx_blk, num_idxs_reg=nidx_blk,
                        elem_size=FTAB, single_packet=False)
                    g2 = workp.tile([128, slots, 64], f32, tag="g2")
                    nc.gpsimd.dma_gather(
                        g2[:], t1[:, 0:64], dstw[:, b * IW:(b + 1) * IW],
                        num_idxs=nidx_blk, num_idxs_reg=nidx_blk,
                        elem_size=64, elem_step=FTAB, single_packet=False)

                    # ---- attention logits -> ex ----
                    ex = workp.tile([128, slots, H], f32, tag="ex")
                    nc.vector.tensor_tensor(ex[:], g1[:, :, 4:8], g2[:, :, 0:4],
                                            mybir.AluOpType.add)
                    nc.vector.scalar_tensor_tensor(ex[:], ex[:], NEG, ex[:],
                                                   mybir.AluOpType.mult,
                                                   mybir.AluOpType.max)
                    nc.scalar.activation(ex[:], ex[:], mybir.ActivationFunctionType.Exp)

                    blk_ps = psump.tile([128, 4 + HID], f32, tag="blk_ps")
                    for s in range(slots):
                        # one-hot scatter matrix for this slot
                        m01 = slotp.tile([128, 128], f32, tag="m01")
                        nc.vector.tensor_scalar(
                            m01[:], iota[:], dloc[:, b * slots + s:b * slots + s + 1], None,
                            mybir.AluOpType.is_equal)
                        # scaled messages [ex | ex*xp]
                        gs = slotp.tile([128, 4 + HID], f32, tag="gs")
                        nc.vector.tensor_copy(gs[:, 0:4], ex[:, s, :])
                        for h in range(H):
                            dst_sl = gs[:, 4 + h * C:4 + (h + 1) * C]
                            src_sl = g1[:, s, 8 + h * C:8 + (h + 1) * C]
                            if h % 2 == 0:
                                nc.scalar.activation(
                                    dst_sl, src_sl,
                                    mybir.ActivationFunctionType.Copy,
                                    scale=ex[:, s, h:h + 1])
                            else:
                                nc.vector.tensor_scalar(
                                    dst_sl, src_sl, ex[:, s, h:h + 1], None,
                                    mybir.AluOpType.mult)
                        nc.tensor.matmul(blk_ps[:], m01[:], gs[:],
                                         start=(s == 0), stop=(s == slots - 1))

                    # ---- normalize by denom, BN + ReLU ----
                    rec = smallp.tile([128, H], f32, tag="rec")
                    nc.vector.tensor_scalar(rec[:w, :], blk_ps[:w, 0:4], 1e-16, None,
                                            mybir.AluOpType.add)
                    nc.vector.reciprocal(rec[:w, :], rec[:w, :])
                    hb = smallp.tile([128, HID], f32, tag="hb")
                    for h in range(H):
                        nc.vector.tensor_scalar(
                            hb[:w, h * C:(h + 1) * C],
                            blk_ps[:w, 4 + h * C:4 + (h + 1) * C],
                            rec[:w, h:h + 1], None, mybir.AluOpType.mult)
                    nc.vector.tensor_tensor(hb[:w, :], hb[:w, :],
                                            sb[:w, layer * HID:(layer + 1) * HID],
                                            mybir.AluOpType.mult)
                    nc.vector.tensor_tensor(hb[:w, :], hb[:w, :],
                                            shb[:w, layer * HID:(layer + 1) * HID],
                                            mybir.AluOpType.add)
                    nc.scalar.activation(hb[:w, :], hb[:w, :],
                                         mybir.ActivationFunctionType.Relu)

                    if layer < NL - 1:
                        # ---- transpose h, next-layer table rows ----
                        for k in range(2):
                            trp = psump.tile([128, 128], f32, tag="trp")
                            nc.tensor.transpose(trp[:], hb[:, k * 128:(k + 1) * 128],
                                                ident[:])
                            hT = hT0 if k == 0 else hT1
                            nc.scalar.copy(hT[:, b * BLK:b * BLK + w], trp[:, :w])
                        xp_ps = psump.tile([128, ROWW], f32, tag="xp_ps")
                        for k in range(2):
                            hT = hT0 if k == 0 else hT1
                            nc.tensor.matmul(xp_ps[:w, :],
                                             hT[:, b * BLK:b * BLK + w],
                                             wc[:, (layer * 2 + k) * ROWW:(layer * 2 + k + 1) * ROWW],
                                             start=(k == 0), stop=(k == 1))
                        row = smallp.tile([128, ROWW], f32, tag="tabrow")
                        nc.scalar.copy(row[:w, :], xp_ps[:w, :])
                        nc.sync.dma_start(agin[b * BLK:b * BLK + w, 0:ROWW],
                                          row[:w, :])
                    else:
                        # ---- pooling ----
                        mp = slotp.tile([128, G], f32, tag="mpool")
                        nc.vector.tensor_scalar(
                            mp[:], iota[:, 0:G], poolv[:, b:b + 1], None,
                            mybir.AluOpType.is_equal)
                        nc.tensor.matmul(pool_ps[:], mp[:], hb[:],
                                         start=(b == 0), stop=(b == NBLK - 1),
                                         skip_group_check=True)

            pout = smallp.tile([G, HID], f32, tag="pout")
            nc.scalar.copy(pout[:], pool_ps[:])
            nc.sync.dma_start(pool_d[:], pout[:])

    nc.compile()
    return nc


def _host_inputs(inputs, srcw, dstw, dl, poolv):
    x = np.asarray(inputs["x"], dtype=np.float32)
    W0 = np.asarray(inputs["W0"], dtype=np.float32)
    W_rest = np.asarray(inputs["W_rest"], dtype=np.float32)
    att_src = np.asarray(inputs["att_src"], dtype=np.float32)
    att_dst = np.asarray(inputs["att_dst"], dtype=np.float32)
    bias_conv = np.asarray(inputs["bias_conv"], dtype=np.float32)
    bn_gamma = np.asarray(inputs["bn_gamma"], dtype=np.float32)
    bn_beta = np.asarray(inputs["bn_beta"], dtype=np.float32)
    bn_mean = np.asarray(inputs["bn_mean"], dtype=np.float32)
    bn_var = np.asarray(inputs["bn_var"], dtype=np.float32)

    wcats = []
    for layer in range(NL):
        Wl = W0 if layer == 0 else W_rest[layer - 1]
        Ad = _amat(att_dst[layer])
        As = _amat(att_src[layer])
        wcat = np.concatenate([Wl @ Ad, Wl @ As, Wl], axis=1)  # [in, 264]
        wcats.append(wcat.astype(np.float32))
    w0cat = wcats[0]
    # [128, (NL-1)*2*ROWW]: col block (layer*2+k) holds W-chunk k of layer+1
    wcat = np.concatenate(
        [w.reshape(2, 128, ROWW)[k] for w in wcats[1:] for k in range(2)], axis=1)
    wcat = np.ascontiguousarray(wcat)

    s = bn_gamma / np.sqrt(bn_var + BN_EPS)            # [NL, HID]
    shift = (bias_conv - bn_mean) * s + bn_beta
    sb = np.ascontiguousarray(np.broadcast_to(s.reshape(-1), (128, NL * HID)))
    shb = np.ascontiguousarray(np.broadcast_to(shift.reshape(-1), (128, NL * HID)))

    iota = np.broadcast_to(np.arange(128, dtype=np.float32), (128, 128))
    iota = np.ascontiguousarray(iota)
    ident = np.eye(128, dtype=np.float32)

    in_maps = []
    for c in range(NCORE):
        xs = x[c * NSH:(c + 1) * NSH, :]   # [NSH, IN]
        in_maps.append(dict(
            xT=np.ascontiguousarray(xs.T),
            w0cat=w0cat, wcat=wcat, sb=sb, shb=shb,
            iota=iota, ident=ident,
            srcw=srcw[c], dstw=dstw[c], dloc=dl[c], poolv=poolv[c],
        ))
    return in_maps


def _postprocess(partials, inputs):
    pooled = np.sum(np.stack(partials), axis=0)  # [G, HID]
    lg = np.asarray(inputs["lbn_gamma"], dtype=np.float32)
    lb = np.asarray(inputs["lbn_beta"], dtype=np.float32)
    lm = np.asarray(inputs["lbn_mean"], dtype=np.float32)
    lv = np.asarray(inputs["lbn_var"], dtype=np.float32)
    fw = np.asarray(inputs["fc_W"], dtype=np.float32)
    fb = np.asarray(inputs["fc_b"], dtype=np.float32)
    pooled = (pooled - lm) / np.sqrt(lv + BN_EPS) * lg + lb
    return (pooled @ fw + fb).astype(np.float32)


def _get_program(slots):
    key = ("prog", slots)
    if key not in _cache:
        _cache[key] = _build(slots)
    return _cache[key]


def run(inputs, trace=False, trace_kwargs=None):
    from concourse.bass_utils import run_bass_kernel_spmd
    srcw, dstw, dl, poolv, slots = _preprocess(inputs["edge_index"], inputs["batch"])
    nc = _get_program(slots)
    in_maps = _host_inputs(inputs, srcw, dstw, dl, poolv)
    res = run_bass_kernel_spmd(nc, in_maps, list(range(NCORE)),
                               trace=trace, **(trace_kwargs or {}))
    partials = [res.results[c]["pool_out"] for c in range(NCORE)]
    return _postprocess(partials, inputs), res


def kernel(**inputs) -> np.ndarray:
    out, _ = run(inputs)
    return out


# revision 10
# speedup vs baseline: 1.6482x; 1.6482x over previous
"""3-layer GAT + global pool on 8 Trainium2 NeuronCores (Bass/Tile SPMD).

Strategy: shard edges by destination node across the 8 cores (2500 dst
nodes each). Per layer, each core computes the node table
[al_d | al_s | xp] for its node shard; two AllGather collectives (split
in halves so the first overlaps trailing compute) replicate the full
table to Shared DRAM on every core. All per-edge work is core-local:
one dma_gather by src per 128-dst-node block, attention coefficients
exp(leakyrelu(al_s[src]+al_d[dst])) with the dst-side broadcast done by
a transposed one-hot matmul (no per-edge gather), and the weighted
scatter-add done as one-hot matmuls accumulating [denom | sum ex*xp] in
PSUM. Final pooled [64, HID] partials are summed on the host, which
also applies the (tiny) final BatchNorm + FC.
"""
import numpy as np

# ---- model constants (must match the reference) ----
N = 20000
E = 320000
G = 64
H = 4
C = 64
HID = H * C          # 256
IN = 128
LAT = 64
NEG = 0.2
BN_EPS = 1e-5
NL = 3

NCORE = 8
NSH = N // NCORE     # 2500 dst nodes per core
BLK = 128
NBLK = (NSH + BLK - 1) // BLK   # 20 (last block 68 nodes)
NBLK_A = NBLK // 2              # blocks in the first AllGather half
FTAB = 320           # table row: [al_d(4) | al_s(4) | xp(256) | pad(56)]
ROWW = 8 + HID       # 264 useful cols

_cache = {}


def _halves():
    na = min(NBLK_A * BLK, NSH)   # rows per core in half a
    nb = NSH - na
    return na, nb


def _rowmap():
    """node id -> row id in the split-AllGather table layout."""
    na, nb = _halves()
    n = np.arange(N)
    r = n // NSH
    i = n % NSH
    return np.where(i < na, r * na + i, NCORE * na + r * nb + (i - na))


def _wrap16(idx):
    """[..., NIDX] int -> [..., 128, NIDX//16] int16 wrapped layout:
    element i at [i%16, i//16], replicated across the 8 groups of 16."""
    lead = idx.shape[:-1]
    nidx = idx.shape[-1]
    t = idx.reshape(*lead, nidx // 16, 16)
    t = np.swapaxes(t, -1, -2)  # [..., 16, nidx//16]
    out = np.broadcast_to(t[..., None, :, :], (*lead, 8, 16, nidx // 16))
    return np.ascontiguousarray(out.reshape(*lead, 128, nidx // 16)).astype(np.int16)


def _preprocess(edge_index, batch):
    src = np.concatenate([np.asarray(edge_index[0]), np.arange(N)]).astype(np.int64)
    dst = np.concatenate([np.asarray(edge_index[1]), np.arange(N)]).astype(np.int64)
    order = np.argsort(dst, kind="stable")
    srcs = src[order]
    dsts = dst[order]

    bounds = []
    for c in range(NCORE):
        for b in range(NBLK):
            bounds.append(c * NSH + b * BLK)
    bounds.append(N)
    cuts = np.searchsorted(dsts, np.array(bounds))
    cnts = np.diff(cuts)
    slots = int(np.ceil(cnts.max() / BLK))
    nidx_blk = slots * BLK

    rowmap = _rowmap()
    srcpad = np.zeros((NCORE, NBLK, nidx_blk), dtype=np.int64)
    dloc = np.full((NCORE, NBLK, nidx_blk), -1.0, dtype=np.float32)
    for c in range(NCORE):
        for b in range(NBLK):
            k = c * NBLK + b
            lo, hi = cuts[k], cuts[k + 1]
            n = hi - lo
            srcpad[c, b, :n] = rowmap[srcs[lo:hi]]
            dloc[c, b, :n] = (dsts[lo:hi] - (c * NSH + b * BLK)).astype(np.float32)

    srcw = _wrap16(srcpad.reshape(NCORE, NBLK * nidx_blk))  # [NC,128,NBLK*nidx/16]
    # slot-major dloc for the M01 build: [p, b*SLOTS+s] = dloc[b, s*128+p]
    dl = dloc.reshape(NCORE, NBLK, slots, BLK).transpose(0, 3, 1, 2)
    dl = np.ascontiguousarray(dl.reshape(NCORE, BLK, NBLK * slots))
    # edge-major dloc for the M01T build (partition-broadcast per block)
    dlT = np.ascontiguousarray(dloc.reshape(NCORE, NBLK, nidx_blk))

    bat = np.asarray(batch).astype(np.int64)
    poolv = np.full((NCORE, BLK, NBLK), -1.0, dtype=np.float32)
    for c in range(NCORE):
        for b in range(NBLK):
            lo = c * NSH + b * BLK
            n = min(BLK, c * NSH + NSH - lo)
            poolv[c, :n, b] = bat[lo:lo + n].astype(np.float32)
    return srcw, dl, dlT, poolv, slots


def _amat(att):  # att [H, C] -> [HID, H] block diagonal
    A = np.zeros((H, C, H), dtype=np.float32)
    for h in range(H):
        A[h, :, h] = att[h]
    return A.reshape(HID, H)


def _build(slots):
    import concourse.bacc as bacc
    import concourse.mybir as mybir
    import concourse.tile as tile

    f32 = mybir.dt.float32
    i16 = mybir.dt.int16
    nidx_blk = slots * BLK
    na, nb = _halves()

    nc = bacc.Bacc("TRN2", target_bir_lowering=False, debug=False,
                   enable_asserts=True, num_devices=NCORE)

    xT_d = nc.dram_tensor("xT", [IN, NSH], f32, kind="ExternalInput")
    w0_d = nc.dram_tensor("w0cat", [IN, ROWW], f32, kind="ExternalInput")
    wc_d = nc.dram_tensor("wcat", [128, (NL - 1) * 2 * ROWW], f32, kind="ExternalInput")
    sb_d = nc.dram_tensor("sb", [128, NL * HID], f32, kind="ExternalInput")
    shb_d = nc.dram_tensor("shb", [128, NL * HID], f32, kind="ExternalInput")
    iota_d = nc.dram_tensor("iota", [128, 128], f32, kind="ExternalInput")
    iotac_d = nc.dram_tensor("iotac", [128, 1], f32, kind="ExternalInput")
    iotas_d = nc.dram_tensor("iotas", [128, slots * 128], f32, kind="ExternalInput")
    ident_d = nc.dram_tensor("ident", [128, 128], f32, kind="ExternalInput")
    srcw_d = nc.dram_tensor("srcw", [128, NBLK * (nidx_blk // 16)], i16, kind="ExternalInput")
    dloc_d = nc.dram_tensor("dloc", [128, NBLK * slots], f32, kind="ExternalInput")
    dlT_d = nc.dram_tensor("dlT", [NBLK, nidx_blk], f32, kind="ExternalInput")
    poolv_d = nc.dram_tensor("poolv", [128, NBLK], f32, kind="ExternalInput")

    pool_d = nc.dram_tensor("pool_out", [G, HID], f32, kind="ExternalOutput")

    IW = nidx_blk // 16

    with tile.TileContext(nc) as tc:
        with (
            tc.tile_pool(name="const", bufs=1) as constp,
            tc.tile_pool(name="work", bufs=2) as workp,
            tc.tile_pool(name="small", bufs=3) as smallp,
            tc.tile_pool(name="psum", bufs=2, space="PSUM") as psump,
            tc.tile_pool(name="psed", bufs=2, space="PSUM") as psedp,
            tc.tile_pool(name="pstr", bufs=1, space="PSUM") as pstrp,
            tc.tile_pool(name="psacc", bufs=1, space="PSUM") as psaccp,
            tc.tile_pool(name="dram", bufs=1, space="DRAM") as dramp,
        ):
            # ---- constants ----
            xT = constp.tile([IN, NSH], f32, tag="xT")
            nc.sync.dma_start(xT[:], xT_d[:])
            w0 = constp.tile([IN, ROWW], f32, tag="w0")
            nc.sync.dma_start(w0[:], w0_d[:])
            wc = constp.tile([128, (NL - 1) * 2 * ROWW], f32, tag="wc")
            nc.sync.dma_start(wc[:], wc_d[:])
            sb = constp.tile([128, NL * HID], f32, tag="sb")
            nc.sync.dma_start(sb[:], sb_d[:])
            shb = constp.tile([128, NL * HID], f32, tag="shb")
            nc.sync.dma_start(shb[:], shb_d[:])
            iota = constp.tile([128, 128], f32, tag="iota")
            nc.sync.dma_start(iota[:], iota_d[:])
            iota_c = constp.tile([128, 1], f32, tag="iotac")
            nc.sync.dma_start(iota_c[:], iotac_d[:])
            iota_s = constp.tile([128, slots, 128], f32, tag="iotas")
            nc.sync.dma_start(iota_s[:].rearrange("p s d -> p (s d)"), iotas_d[:])
            ident = constp.tile([128, 128], f32, tag="ident")
            nc.sync.dma_start(ident[:], ident_d[:])
            srcw = constp.tile([128, NBLK * IW], i16, tag="srcw")
            nc.sync.dma_start(srcw[:], srcw_d[:])
            dloc = constp.tile([128, NBLK * slots], f32, tag="dloc")
            nc.sync.dma_start(dloc[:], dloc_d[:])
            poolv = constp.tile([128, NBLK], f32, tag="poolv")
            nc.sync.dma_start(poolv[:], poolv_d[:])

            hT0 = constp.tile([128, NSH], f32, tag="hT0")
            hT1 = constp.tile([128, NSH], f32, tag="hT1")
            alds = [constp.tile([128, NBLK * H], f32, tag=f"ald{l}",
                                name=f"ald{l}")
                    for l in range(NL)]

            pool_ps = psaccp.tile([G, HID], f32, tag="poolps")

            t1s = [tc.tile([N, FTAB], f32, space="DRAM", addr_space="Shared",
                           name=f"t1_{l}")[0] for l in range(NL)]
            agins = [(dramp.tile([na, FTAB], f32, tag=f"agin{l}a",
                                 name=f"agin{l}a"),
                      dramp.tile([nb, FTAB], f32, tag=f"agin{l}b",
                                 name=f"agin{l}b"))
                     for l in range(NL)]

            def blkw(b):
                return min(BLK, NSH - b * BLK)

            def write_tabrow(lnext, b, ps, w):
                """psum [w, ROWW] -> AllGather input rows + local al_d stash."""
                row = smallp.tile([128, ROWW], f32, tag="tabrow")
                nc.scalar.copy(row[:w, :], ps[:w, :])
                nc.vector.tensor_copy(alds[lnext][:w, b * H:(b + 1) * H],
                                      ps[:w, 0:H])
                aga, agb = agins[lnext]
                if b < NBLK_A:
                    dst = aga[b * BLK:b * BLK + w, 0:ROWW]
                else:
                    lo = b * BLK - na
                    dst = agb[lo:lo + w, 0:ROWW]
                nc.sync.dma_start(dst, row[:w, :])

            def ag_half(lnext, half):
                aga, agb = agins[lnext]
                src = aga if half == 0 else agb
                rows = na if half == 0 else nb
                off = 0 if half == 0 else NCORE * na
                out = t1s[lnext][off:off + NCORE * rows, :]
                nc.gpsimd.collective_compute(
                    "AllGather", mybir.AluOpType.bypass,
                    ins=[src.opt()], outs=[out.opt()],
                    replica_groups=[list(range(NCORE))],
                )

            # ---- layer-1 table from x ----
            for b in range(NBLK):
                w = blkw(b)
                ps = psump.tile([128, ROWW], f32, tag="xp_ps")
                nc.tensor.matmul(ps[:w, :], xT[:, b * BLK:b * BLK + w], w0[:],
                                 start=True, stop=True)
                write_tabrow(0, b, ps, w)
                if b == NBLK_A - 1:
                    ag_half(0, 0)
            ag_half(0, 1)

            for layer in range(NL):
                t1 = t1s[layer]
                for b in range(NBLK):
                    w = blkw(b)
                    # ---- gather xp+al_s of edge sources ----
                    g1 = workp.tile([128, slots, FTAB], f32, tag="g1")
                    nc.gpsimd.dma_gather(
                        g1[:], t1[:], srcw[:, b * IW:(b + 1) * IW],
                        num_idxs=nidx_blk, num_idxs_reg=nidx_blk,
                        elem_size=FTAB, single_packet=False)

                    # ---- M01T and al_d -> per-edge broadcast ----
                    dlt = workp.tile([128, nidx_blk], f32, tag="dlt")
                    nc.sync.dma_start(
                        dlt[:], dlT_d[b:b + 1, :].partition_broadcast(128))
                    m01t = workp.tile([128, nidx_blk], f32, tag="m01t")
                    nc.vector.tensor_scalar(m01t[:], dlt[:], iota_c[:], None,
                                            mybir.AluOpType.is_equal)
                    ed_ps = psedp.tile([128, slots * H], f32, tag="ed_ps")
                    for s in range(slots):
                        nc.tensor.matmul(
                            ed_ps[:, s * H:(s + 1) * H],
                            m01t[:, s * BLK:(s + 1) * BLK],
                            alds[layer][:, b * H:(b + 1) * H],
                            start=True, stop=True)

                    # ---- ex = exp(leakyrelu(al_s + al_d)), scaled messages --
                    gs = workp.tile([128, slots, H + HID], f32, tag="gs")
                    exv = gs[:, :, 0:H]
                    nc.vector.tensor_tensor(
                        exv, g1[:, :, H:2 * H],
                        ed_ps[:].rearrange("p (s h) -> p s h", h=H),
                        mybir.AluOpType.add)
                    nc.vector.scalar_tensor_tensor(exv, exv, NEG, exv,
                                                   mybir.AluOpType.mult,
                                                   mybir.AluOpType.max)
                    nc.scalar.activation(exv, exv, mybir.ActivationFunctionType.Exp)
                    nc.vector.tensor_tensor(
                        gs[:, :, H:H + HID].rearrange("p s (h c) -> p s h c", c=C),
                        g1[:, :, 2 * H:2 * H + HID].rearrange("p s (h c) -> p s h c", c=C),
                        exv.broadcast_to([128, slots, H, C]),
                        mybir.AluOpType.mult)
                    m01 = workp.tile([128, slots, 128], f32, tag="m01")
                    nc.vector.tensor_tensor(
                        m01[:], iota_s[:],
                        dloc[:, b * slots:(b + 1) * slots].broadcast_to([128, slots, 128]),
                        mybir.AluOpType.is_equal)

                    blk_ps = psump.tile([128, H + HID], f32, tag="blk_ps")
                    for s in range(slots):
                        nc.tensor.matmul(blk_ps[:], m01[:, s, :], gs[:, s, :],
                                         start=(s == 0), stop=(s == slots - 1))

                    # ---- normalize, BN + ReLU ----
                    rec = smallp.tile([128, H], f32, tag="rec")
                    nc.vector.tensor_scalar(rec[:w, :], blk_ps[:w, 0:H], 1e-16,
                                            None, mybir.AluOpType.add)
                    nc.vector.reciprocal(rec[:w, :], rec[:w, :])
                    hb = smallp.tile([128, HID], f32, tag="hb")
                    nc.vector.tensor_tensor(
                        hb[:w, :].rearrange("p (h c) -> p h c", c=C),
                        blk_ps[:w, H:H + HID].rearrange("p (h c) -> p h c", c=C),
                        rec[:w, :].broadcast_to([w, H, C]),
                        mybir.AluOpType.mult)
                    nc.vector.tensor_tensor(hb[:w, :], hb[:w, :],
                                            sb[:w, layer * HID:(layer + 1) * HID],
                                            mybir.AluOpType.mult)
                    nc.vector.tensor_tensor(hb[:w, :], hb[:w, :],
                                            shb[:w, layer * HID:(layer + 1) * HID],
                                            mybir.AluOpType.add)
                    nc.scalar.activation(hb[:w, :], hb[:w, :],
                                         mybir.ActivationFunctionType.Relu)

                    if layer < NL - 1:
                        for k in range(2):
                            trp = pstrp.tile([128, 128], f32, tag="trp")
                            nc.tensor.transpose(trp[:], hb[:, k * 128:(k + 1) * 128],
                                                ident[:])
                            hT = hT0 if k == 0 else hT1
                            nc.scalar.copy(hT[:, b * BLK:b * BLK + w], trp[:, :w])
                        xp_ps = psump.tile([128, ROWW], f32, tag="xp_ps")
                        for k in range(2):
                            hT = hT0 if k == 0 else hT1
                            nc.tensor.matmul(
                                xp_ps[:w, :], hT[:, b * BLK:b * BLK + w],
                                wc[:, (layer * 2 + k) * ROWW:(layer * 2 + k + 1) * ROWW],
                                start=(k == 0), stop=(k == 1))
                        write_tabrow(layer + 1, b, xp_ps, w)
                        if b == NBLK_A - 1:
                            ag_half(layer + 1, 0)
                    else:
                        mp = smallp.tile([128, G], f32, tag="mpool")
                        nc.vector.tensor_scalar(
                            mp[:], iota[:, 0:G], poolv[:, b:b + 1], None,
                            mybir.AluOpType.is_equal)
                        nc.tensor.matmul(pool_ps[:], mp[:], hb[:],
                                         start=(b == 0), stop=(b == NBLK - 1),
                                         skip_group_check=True)
                if layer < NL - 1:
                    ag_half(layer + 1, 1)

            pout = smallp.tile([G, HID], f32, tag="pout")
            nc.scalar.copy(pout[:], pool_ps[:])
            nc.sync.dma_start(pool_d[:], pout[:])

    nc.compile()
    return nc


def _host_inputs(inputs, srcw, dl, dlT, poolv, slots):
    x = np.asarray(inputs["x"], dtype=np.float32)
    W0 = np.asarray(inputs["W0"], dtype=np.float32)
    W_rest = np.asarray(inputs["W_rest"], dtype=np.float32)
    att_src = np.asarray(inputs["att_src"], dtype=np.float32)
    att_dst = np.asarray(inputs["att_dst"], dtype=np.float32)
    bias_conv = np.asarray(inputs["bias_conv"], dtype=np.float32)
    bn_gamma = np.asarray(inputs["bn_gamma"], dtype=np.float32)
    bn_beta = np.asarray(inputs["bn_beta"], dtype=np.float32)
    bn_mean = np.asarray(inputs["bn_mean"], dtype=np.float32)
    bn_var = np.asarray(inputs["bn_var"], dtype=np.float32)

    wcats = []
    for layer in range(NL):
        Wl = W0 if layer == 0 else W_rest[layer - 1]
        Ad = _amat(att_dst[layer])
        As = _amat(att_src[layer])
        wcat = np.concatenate([Wl @ Ad, Wl @ As, Wl], axis=1)  # [in, 264]
        wcats.append(wcat.astype(np.float32))
    w0cat = wcats[0]
    wcat = np.concatenate(
        [w.reshape(2, 128, ROWW)[k] for w in wcats[1:] for k in range(2)], axis=1)
    wcat = np.ascontiguousarray(wcat)

    s = bn_gamma / np.sqrt(bn_var + BN_EPS)
    shift = (bias_conv - bn_mean) * s + bn_beta
    sb = np.ascontiguousarray(np.broadcast_to(s.reshape(-1), (128, NL * HID)))
    shb = np.ascontiguousarray(np.broadcast_to(shift.reshape(-1), (128, NL * HID)))

    iota = np.ascontiguousarray(
        np.broadcast_to(np.arange(128, dtype=np.float32), (128, 128)))
    iotac = np.arange(128, dtype=np.float32).reshape(128, 1)
    iotas = np.ascontiguousarray(np.broadcast_to(
        np.tile(np.arange(128, dtype=np.float32), slots), (128, slots * 128)))
    ident = np.eye(128, dtype=np.float32)

    in_maps = []
    for c in range(NCORE):
        xs = x[c * NSH:(c + 1) * NSH, :]
        in_maps.append(dict(
            xT=np.ascontiguousarray(xs.T),
            w0cat=w0cat, wcat=wcat, sb=sb, shb=shb,
            iota=iota, iotac=iotac, iotas=iotas, ident=ident,
            srcw=srcw[c], dloc=dl[c], dlT=dlT[c], poolv=poolv[c],
        ))
    return in_maps


def _postprocess(partials, inputs):
    pooled = np.sum(np.stack(partials), axis=0)
    lg = np.asarray(inputs["lbn_gamma"], dtype=np.float32)
    lb = np.asarray(inputs["lbn_beta"], dtype=np.float32)
    lm = np.asarray(inputs["lbn_mean"], dtype=np.float32)
    lv = np.asarray(inputs["lbn_var"], dtype=np.float32)
    fw = np.asarray(inputs["fc_W"], dtype=np.float32)
    fb = np.asarray(inputs["fc_b"], dtype=np.float32)
    pooled = (pooled - lm) / np.sqrt(lv + BN_EPS) * lg + lb
    return (pooled @ fw + fb).astype(np.float32)


def _get_program(slots):
    key = ("prog", slots)
    if key not in _cache:
        _cache[key] = _build(slots)
    return _cache[key]


def run(inputs, trace=False, trace_kwargs=None):
    from concourse.bass_utils import run_bass_kernel_spmd
    srcw, dl, dlT, poolv, slots = _preprocess(inputs["edge_index"], inputs["batch"])
    nc = _get_program(slots)
    in_maps = _host_inputs(inputs, srcw, dl, dlT, poolv, slots)
    res = run_bass_kernel_spmd(nc, in_maps, list(range(NCORE)),
                               trace=trace, **(trace_kwargs or {}))
    partials = [res.results[c]["pool_out"] for c in range(NCORE)]
    return _postprocess(partials, inputs), res


def kernel(**inputs) -> np.ndarray:
    out, _ = run(inputs)
    return out


# revision 12
# speedup vs baseline: 1.7902x; 1.0861x over previous
"""3-layer GAT + global pool on 8 Trainium2 NeuronCores (Bass/Tile SPMD).

Strategy: shard edges by destination node across the 8 cores (2500 dst
nodes each). Per layer, each core computes the node table
[al_d | al_s | xp] for its node shard; two AllGather collectives (split
in halves so the first overlaps trailing compute) replicate the full
table to Shared DRAM on every core. All per-edge work is core-local:
one dma_gather by src per 128-dst-node block, attention coefficients
exp(leakyrelu(al_s[src]+al_d[dst])) with the dst-side broadcast done by
a transposed one-hot matmul (no per-edge gather), and the weighted
scatter-add done as one-hot matmuls accumulating [denom | sum ex*xp] in
PSUM. Final pooled [64, HID] partials are summed on the host, which
also applies the (tiny) final BatchNorm + FC.
"""
import numpy as np

# ---- model constants (must match the reference) ----
N = 20000
E = 320000
G = 64
H = 4
C = 64
HID = H * C          # 256
IN = 128
LAT = 64
NEG = 0.2
BN_EPS = 1e-5
NL = 3

NCORE = 8
NSH = N // NCORE     # 2500 dst nodes per core
BLK = 128
NBLK = (NSH + BLK - 1) // BLK   # 20 (last block 68 nodes)
NBLK_A = max(1, NBLK - 2)       # big first AllGather half (overlaps compute)
FTAB = 320           # table row: [al_d(4) | al_s(4) | xp(256) | pad(56)]
ROWW = 8 + HID       # 264 useful cols

_cache = {}


def _halves():
    na = min(NBLK_A * BLK, NSH)   # rows per core in half a
    nb = NSH - na
    return na, nb


def _rowmap():
    """node id -> row id in the split-AllGather table layout."""
    na, nb = _halves()
    n = np.arange(N)
    r = n // NSH
    i = n % NSH
    return np.where(i < na, r * na + i, NCORE * na + r * nb + (i - na))


def _wrap16(idx):
    """[..., NIDX] int -> [..., 128, NIDX//16] int16 wrapped layout:
    element i at [i%16, i//16], replicated across the 8 groups of 16."""
    lead = idx.shape[:-1]
    nidx = idx.shape[-1]
    t = idx.reshape(*lead, nidx // 16, 16)
    t = np.swapaxes(t, -1, -2)  # [..., 16, nidx//16]
    out = np.broadcast_to(t[..., None, :, :], (*lead, 8, 16, nidx // 16))
    return np.ascontiguousarray(out.reshape(*lead, 128, nidx // 16)).astype(np.int16)


def _preprocess(edge_index, batch):
    src = np.concatenate([np.asarray(edge_index[0]), np.arange(N)]).astype(np.int64)
    dst = np.concatenate([np.asarray(edge_index[1]), np.arange(N)]).astype(np.int64)
    order = np.argsort(dst, kind="stable")
    srcs = src[order]
    dsts = dst[order]

    bounds = []
    for c in range(NCORE):
        for b in range(NBLK):
            bounds.append(c * NSH + b * BLK)
    bounds.append(N)
    cuts = np.searchsorted(dsts, np.array(bounds))
    cnts = np.diff(cuts)
    slots = int(np.ceil(cnts.max() / BLK))
    nidx_blk = slots * BLK

    rowmap = _rowmap()
    srcpad = np.zeros((NCORE, NBLK, nidx_blk), dtype=np.int64)
    dloc = np.full((NCORE, NBLK, nidx_blk), -1.0, dtype=np.float32)
    for c in range(NCORE):
        for b in range(NBLK):
            k = c * NBLK + b
            lo, hi = cuts[k], cuts[k + 1]
            n = hi - lo
            srcpad[c, b, :n] = rowmap[srcs[lo:hi]]
            dloc[c, b, :n] = (dsts[lo:hi] - (c * NSH + b * BLK)).astype(np.float32)

    srcw = _wrap16(srcpad.reshape(NCORE, NBLK * nidx_blk))  # [NC,128,NBLK*nidx/16]
    # slot-major dloc for the M01 build: [p, b*SLOTS+s] = dloc[b, s*128+p]
    dl = dloc.reshape(NCORE, NBLK, slots, BLK).transpose(0, 3, 1, 2)
    dl = np.ascontiguousarray(dl.reshape(NCORE, BLK, NBLK * slots))
    # edge-major dloc for the M01T build (partition-broadcast per block)
    dlT = np.ascontiguousarray(dloc.reshape(NCORE, NBLK, nidx_blk))

    bat = np.asarray(batch).astype(np.int64)
    poolv = np.full((NCORE, BLK, NBLK), -1.0, dtype=np.float32)
    for c in range(NCORE):
        for b in range(NBLK):
            lo = c * NSH + b * BLK
            n = min(BLK, c * NSH + NSH - lo)
            poolv[c, :n, b] = bat[lo:lo + n].astype(np.float32)
    return srcw, dl, dlT, poolv, slots


def _amat(att):  # att [H, C] -> [HID, H] block diagonal
    A = np.zeros((H, C, H), dtype=np.float32)
    for h in range(H):
        A[h, :, h] = att[h]
    return A.reshape(HID, H)


def _build(slots):
    import concourse.bacc as bacc
    import concourse.mybir as mybir
    import concourse.tile as tile

    f32 = mybir.dt.float32
    bf16 = mybir.dt.bfloat16
    i16 = mybir.dt.int16
    nidx_blk = slots * BLK
    na, nb = _halves()

    nc = bacc.Bacc("TRN2", target_bir_lowering=False, debug=False,
                   enable_asserts=True, num_devices=NCORE)

    xT_d = nc.dram_tensor("xT", [IN, NSH], f32, kind="ExternalInput")
    w0_d = nc.dram_tensor("w0cat", [IN, ROWW], f32, kind="ExternalInput")
    wc_d = nc.dram_tensor("wcat", [128, (NL - 1) * 2 * ROWW], f32, kind="ExternalInput")
    sb_d = nc.dram_tensor("sb", [128, NL * HID], f32, kind="ExternalInput")
    shb_d = nc.dram_tensor("shb", [128, NL * HID], f32, kind="ExternalInput")
    iota_d = nc.dram_tensor("iota", [128, 128], f32, kind="ExternalInput")
    iotac_d = nc.dram_tensor("iotac", [128, 1], f32, kind="ExternalInput")
    iotas_d = nc.dram_tensor("iotas", [128, slots * 128], bf16, kind="ExternalInput")
    ident_d = nc.dram_tensor("ident", [128, 128], f32, kind="ExternalInput")
    srcw_d = nc.dram_tensor("srcw", [128, NBLK * (nidx_blk // 16)], i16, kind="ExternalInput")
    dloc_d = nc.dram_tensor("dloc", [128, NBLK * slots], bf16, kind="ExternalInput")
    dlT_d = nc.dram_tensor("dlT", [NBLK, nidx_blk], bf16, kind="ExternalInput")
    poolv_d = nc.dram_tensor("poolv", [128, NBLK], f32, kind="ExternalInput")

    pool_d = nc.dram_tensor("pool_out", [G, HID], f32, kind="ExternalOutput")

    IW = nidx_blk // 16

    with tile.TileContext(nc) as tc:
        with (
            tc.tile_pool(name="const", bufs=1) as constp,
            tc.tile_pool(name="g1p", bufs=3) as g1p,
            tc.tile_pool(name="work", bufs=2) as workp,
            tc.tile_pool(name="small", bufs=3) as smallp,
            tc.tile_pool(name="psum", bufs=2, space="PSUM") as psump,
            tc.tile_pool(name="psed", bufs=2, space="PSUM") as psedp,
            tc.tile_pool(name="pstr", bufs=1, space="PSUM") as pstrp,
            tc.tile_pool(name="psacc", bufs=1, space="PSUM") as psaccp,
            tc.tile_pool(name="dram", bufs=1, space="DRAM") as dramp,
        ):
            # ---- constants ----
            xT = g1p.tile([IN, NSH], f32, tag="g1", name="xT")
            nc.sync.dma_start(xT[:], xT_d[:])
            w0 = constp.tile([IN, ROWW], f32, tag="w0")
            nc.sync.dma_start(w0[:], w0_d[:])
            wc = constp.tile([128, (NL - 1) * 2 * ROWW], f32, tag="wc")
            nc.sync.dma_start(wc[:], wc_d[:])
            sb = constp.tile([128, NL * HID], f32, tag="sb")
            nc.sync.dma_start(sb[:], sb_d[:])
            shb = constp.tile([128, NL * HID], f32, tag="shb")
            nc.sync.dma_start(shb[:], shb_d[:])
            iota = constp.tile([128, 128], f32, tag="iota")
            nc.sync.dma_start(iota[:], iota_d[:])
            iota_c = constp.tile([128, 1], f32, tag="iotac")
            nc.sync.dma_start(iota_c[:], iotac_d[:])
            iota_s = constp.tile([128, slots, 128], bf16, tag="iotas")
            nc.sync.dma_start(iota_s[:].rearrange("p s d -> p (s d)"), iotas_d[:])
            ident = constp.tile([128, 128], f32, tag="ident")
            nc.sync.dma_start(ident[:], ident_d[:])
            srcw = constp.tile([128, NBLK * IW], i16, tag="srcw")
            nc.sync.dma_start(srcw[:], srcw_d[:])
            dloc = constp.tile([128, NBLK * slots], bf16, tag="dloc")
            nc.sync.dma_start(dloc[:], dloc_d[:])
            poolv = constp.tile([128, NBLK], f32, tag="poolv")
            nc.sync.dma_start(poolv[:], poolv_d[:])

            hT0 = constp.tile([128, NSH], f32, tag="hT0")
            hT1 = constp.tile([128, NSH], f32, tag="hT1")
            alds = [constp.tile([128, NBLK * H], f32, tag=f"ald{l}",
                                name=f"ald{l}")
                    for l in range(NL)]

            pool_ps = psaccp.tile([G, HID], f32, tag="poolps")

            t1s = [tc.tile([N, FTAB], f32, space="DRAM", addr_space="Shared",
                           name=f"t1_{l}")[0] for l in range(NL)]
            agins = [(dramp.tile([na, FTAB], f32, tag=f"agin{l}a",
                                 name=f"agin{l}a"),
                      dramp.tile([nb, FTAB], f32, tag=f"agin{l}b",
                                 name=f"agin{l}b"))
                     for l in range(NL)]

            def blkw(b):
                return min(BLK, NSH - b * BLK)

            def write_tabrow(lnext, b, ps, w):
                """psum [w, ROWW] -> AllGather input rows + local al_d stash."""
                row = smallp.tile([128, ROWW], f32, tag="tabrow")
                nc.scalar.copy(row[:w, :], ps[:w, :])
                nc.vector.tensor_copy(alds[lnext][:w, b * H:(b + 1) * H],
                                      ps[:w, 0:H])
                aga, agb = agins[lnext]
                if b < NBLK_A:
                    dst = aga[b * BLK:b * BLK + w, 0:ROWW]
                else:
                    lo = b * BLK - na
                    dst = agb[lo:lo + w, 0:ROWW]
                nc.sync.dma_start(dst, row[:w, :])

            def ag_half(lnext, half):
                aga, agb = agins[lnext]
                src = aga if half == 0 else agb
                rows = na if half == 0 else nb
                off = 0 if half == 0 else NCORE * na
                out = t1s[lnext][off:off + NCORE * rows, :]
                nc.gpsimd.collective_compute(
                    "AllGather", mybir.AluOpType.bypass,
                    ins=[src.opt()], outs=[out.opt()],
                    replica_groups=[list(range(NCORE))],
                )

            # ---- layer-1 table from x ----
            for b in range(NBLK):
                w = blkw(b)
                ps = psump.tile([128, ROWW], f32, tag="xp_ps")
                nc.tensor.matmul(ps[:w, :], xT[:, b * BLK:b * BLK + w], w0[:],
                                 start=True, stop=True)
                write_tabrow(0, b, ps, w)
                if b == NBLK_A - 1:
                    ag_half(0, 0)
            ag_half(0, 1)

            for layer in range(NL):
                t1 = t1s[layer]
                for b in range(NBLK):
                    w = blkw(b)
                    # ---- gather xp+al_s of edge sources ----
                    g1 = g1p.tile([128, slots, FTAB], f32, tag="g1")
                    nc.gpsimd.dma_gather(
                        g1[:], t1[:], srcw[:, b * IW:(b + 1) * IW],
                        num_idxs=nidx_blk, num_idxs_reg=nidx_blk,
                        elem_size=FTAB, single_packet=False)

                    # ---- M01T and al_d -> per-edge broadcast ----
                    dlt = workp.tile([128, nidx_blk], bf16, tag="dlt")
                    nc.sync.dma_start(
                        dlt[:], dlT_d[b:b + 1, :].partition_broadcast(128))
                    m01t = workp.tile([128, nidx_blk], f32, tag="m01t")
                    nc.vector.tensor_scalar(m01t[:], dlt[:], iota_c[:], None,
                                            mybir.AluOpType.is_equal)
                    ed_ps = psedp.tile([128, slots * H], f32, tag="ed_ps")
                    for s in range(slots):
                        nc.tensor.matmul(
                            ed_ps[:, s * H:(s + 1) * H],
                            m01t[:, s * BLK:(s + 1) * BLK],
                            alds[layer][:, b * H:(b + 1) * H],
                            start=True, stop=True)

                    # ---- ex = exp(leakyrelu(al_s + al_d)), scaled messages --
                    gs = workp.tile([128, slots, H + HID], bf16, tag="gs")
                    ext = workp.tile([128, slots, H], f32, tag="ext")
                    nc.vector.tensor_tensor(
                        ext[:], g1[:, :, H:2 * H],
                        ed_ps[:].rearrange("p (s h) -> p s h", h=H),
                        mybir.AluOpType.add)
                    nc.vector.scalar_tensor_tensor(ext[:], ext[:], NEG, ext[:],
                                                   mybir.AluOpType.mult,
                                                   mybir.AluOpType.max)
                    nc.scalar.activation(ext[:], ext[:],
                                         mybir.ActivationFunctionType.Exp)
                    nc.vector.tensor_copy(gs[:, :, 0:H], ext[:])
                    nc.vector.tensor_tensor(
                        gs[:, :, H:H + HID].rearrange("p s (h c) -> p s h c", c=C),
                        g1[:, :, 2 * H:2 * H + HID].rearrange("p s (h c) -> p s h c", c=C),
                        ext[:].broadcast_to([128, slots, H, C]),
                        mybir.AluOpType.mult)
                    m01 = workp.tile([128, slots, 128], bf16, tag="m01")
                    nc.vector.tensor_tensor(
                        m01[:], iota_s[:],
                        dloc[:, b * slots:(b + 1) * slots].broadcast_to([128, slots, 128]),
                        mybir.AluOpType.is_equal)

                    blk_ps = psump.tile([128, H + HID], f32, tag="blk_ps")
                    for s in range(slots):
                        nc.tensor.matmul(blk_ps[:], m01[:, s, :], gs[:, s, :],
                                         start=(s == 0), stop=(s == slots - 1))

                    # ---- normalize, BN + ReLU ----
                    rec = smallp.tile([128, H], f32, tag="rec")
                    nc.vector.tensor_scalar(rec[:w, :], blk_ps[:w, 0:H], 1e-16,
                                            None, mybir.AluOpType.add)
                    nc.vector.reciprocal(rec[:w, :], rec[:w, :])
                    hb = smallp.tile([128, HID], f32, tag="hb")
                    nc.vector.tensor_tensor(
                        hb[:w, :].rearrange("p (h c) -> p h c", c=C),
                        blk_ps[:w, H:H + HID].rearrange("p (h c) -> p h c", c=C),
                        rec[:w, :].broadcast_to([w, H, C]),
                        mybir.AluOpType.mult)
                    nc.vector.tensor_tensor(hb[:w, :], hb[:w, :],
                                            sb[:w, layer * HID:(layer + 1) * HID],
                                            mybir.AluOpType.mult)
                    nc.vector.tensor_tensor(hb[:w, :], hb[:w, :],
                                            shb[:w, layer * HID:(layer + 1) * HID],
                                            mybir.AluOpType.add)
                    nc.scalar.activation(hb[:w, :], hb[:w, :],
                                         mybir.ActivationFunctionType.Relu)

                    if layer < NL - 1:
                        for k in range(2):
                            trp = pstrp.tile([128, 128], f32, tag="trp")
                            nc.tensor.transpose(trp[:], hb[:, k * 128:(k + 1) * 128],
                                                ident[:])
                            hT = hT0 if k == 0 else hT1
                            nc.scalar.copy(hT[:, b * BLK:b * BLK + w], trp[:, :w])
                        xp_ps = psump.tile([128, ROWW], f32, tag="xp_ps")
                        for k in range(2):
                            hT = hT0 if k == 0 else hT1
                            nc.tensor.matmul(
                                xp_ps[:w, :], hT[:, b * BLK:b * BLK + w],
                                wc[:, (layer * 2 + k) * ROWW:(layer * 2 + k + 1) * ROWW],
                                start=(k == 0), stop=(k == 1))
                        write_tabrow(layer + 1, b, xp_ps, w)
                        if b == NBLK_A - 1:
                            ag_half(layer + 1, 0)
                    else:
                        mp = smallp.tile([128, G], f32, tag="mpool")
                        nc.vector.tensor_scalar(
                            mp[:], iota[:, 0:G], poolv[:, b:b + 1], None,
                            mybir.AluOpType.is_equal)
                        nc.tensor.matmul(pool_ps[:], mp[:], hb[:],
                                         start=(b == 0), stop=(b == NBLK - 1),
                                         skip_group_check=True)
                if layer < NL - 1:
                    ag_half(layer + 1, 1)

            pout = smallp.tile([G, HID], f32, tag="pout")
            nc.scalar.copy(pout[:], pool_ps[:])
            nc.sync.dma_start(pool_d[:], pout[:])

    nc.compile()
    return nc


def _host_inputs(inputs, srcw, dl, dlT, poolv, slots):
    x = np.asarray(inputs["x"], dtype=np.float32)
    W0 = np.asarray(inputs["W0"], dtype=np.float32)
    W_rest = np.asarray(inputs["W_rest"], dtype=np.float32)
    att_src = np.asarray(inputs["att_src"], dtype=np.float32)
    att_dst = np.asarray(inputs["att_dst"], dtype=np.float32)
    bias_conv = np.asarray(inputs["bias_conv"], dtype=np.float32)
    bn_gamma = np.asarray(inputs["bn_gamma"], dtype=np.float32)
    bn_beta = np.asarray(inputs["bn_beta"], dtype=np.float32)
    bn_mean = np.asarray(inputs["bn_mean"], dtype=np.float32)
    bn_var = np.asarray(inputs["bn_var"], dtype=np.float32)

    wcats = []
    for layer in range(NL):
        Wl = W0 if layer == 0 else W_rest[layer - 1]
        Ad = _amat(att_dst[layer])
        As = _amat(att_src[layer])
        wcat = np.concatenate([Wl @ Ad, Wl @ As, Wl], axis=1)  # [in, 264]
        wcats.append(wcat.astype(np.float32))
    w0cat = wcats[0]
    wcat = np.concatenate(
        [w.reshape(2, 128, ROWW)[k] for w in wcats[1:] for k in range(2)], axis=1)
    wcat = np.ascontiguousarray(wcat)

    s = bn_gamma / np.sqrt(bn_var + BN_EPS)
    shift = (bias_conv - bn_mean) * s + bn_beta
    sb = np.ascontiguousarray(np.broadcast_to(s.reshape(-1), (128, NL * HID)))
    shb = np.ascontiguousarray(np.broadcast_to(shift.reshape(-1), (128, NL * HID)))

    iota = np.ascontiguousarray(
        np.broadcast_to(np.arange(128, dtype=np.float32), (128, 128)))
    import ml_dtypes
    bf = ml_dtypes.bfloat16
    iotac = np.arange(128, dtype=np.float32).reshape(128, 1)
    iotas = np.ascontiguousarray(np.broadcast_to(
        np.tile(np.arange(128, dtype=bf), slots), (128, slots * 128)))
    ident = np.eye(128, dtype=np.float32)

    in_maps = []
    for c in range(NCORE):
        xs = x[c * NSH:(c + 1) * NSH, :]
        in_maps.append(dict(
            xT=np.ascontiguousarray(xs.T),
            w0cat=w0cat, wcat=wcat, sb=sb, shb=shb,
            iota=iota, iotac=iotac, iotas=iotas, ident=ident,
            srcw=srcw[c], dloc=dl[c].astype(bf), dlT=dlT[c].astype(bf),
            poolv=poolv[c],
        ))
    return in_maps


def _postprocess(partials, inputs):
    pooled = np.sum(np.stack(partials), axis=0)
    lg = np.asarray(inputs["lbn_gamma"], dtype=np.float32)
    lb = np.asarray(inputs["lbn_beta"], dtype=np.float32)
    lm = np.asarray(inputs["lbn_mean"], dtype=np.float32)
    lv = np.asarray(inputs["lbn_var"], dtype=np.float32)
    fw = np.asarray(inputs["fc_W"], dtype=np.float32)
    fb = np.asarray(inputs["fc_b"], dtype=np.float32)
    pooled = (pooled - lm) / np.sqrt(lv + BN_EPS) * lg + lb
    return (pooled @ fw + fb).astype(np.float32)


def _get_program(slots):
    key = ("prog", slots)
    if key not in _cache:
        _cache[key] = _build(slots)
    return _cache[key]


def run(inputs, trace=False, trace_kwargs=None):
    from concourse.bass_utils import run_bass_kernel_spmd
    srcw, dl, dlT, poolv, slots = _preprocess(inputs["edge_index"], inputs["batch"])
    nc = _get_program(slots)
    in_maps = _host_inputs(inputs, srcw, dl, dlT, poolv, slots)
    res = run_bass_kernel_spmd(nc, in_maps, list(range(NCORE)),
                               trace=trace, **(trace_kwargs or {}))
    partials = [res.results[c]["pool_out"] for c in range(NCORE)]
    return _postprocess(partials, inputs), res


def kernel(**inputs) -> np.ndarray:
    out, _ = run(inputs)
    return out


# revision 13
# speedup vs baseline: 1.8596x; 1.0388x over previous
"""3-layer GAT + global pool on 8 Trainium2 NeuronCores (Bass/Tile SPMD).

Strategy: shard edges by destination node across the 8 cores (2500 dst
nodes each). Per layer, each core computes the node table
[al_d | al_s | xp] for its node shard; two AllGather collectives (split
in halves so the first overlaps trailing compute) replicate the full
table to Shared DRAM on every core. All per-edge work is core-local:
one dma_gather by src per 128-dst-node block, attention coefficients
exp(leakyrelu(al_s[src]+al_d[dst])) with the dst-side broadcast done by
a transposed one-hot matmul (no per-edge gather), and the weighted
scatter-add done as one-hot matmuls accumulating [denom | sum ex*xp] in
PSUM. Final pooled [64, HID] partials are summed on the host, which
also applies the (tiny) final BatchNorm + FC.
"""
import numpy as np

# ---- model constants (must match the reference) ----
N = 20000
E = 320000
G = 64
H = 4
C = 64
HID = H * C          # 256
IN = 128
LAT = 64
NEG = 0.2
BN_EPS = 1e-5
NL = 3

NCORE = 8
NSH = N // NCORE     # 2500 dst nodes per core
BLK = 128
NBLK = (NSH + BLK - 1) // BLK   # 20 (last block 68 nodes)
def _chunks():
    "AllGather chunks as (block_lo, block_hi) ranges."
    if NBLK >= 20:
        bs = [(0, 8), (8, 16), (16, 19), (19, 20)]
    elif NBLK >= 2:
        bs = [(0, NBLK - 1), (NBLK - 1, NBLK)]
    else:
        bs = [(0, NBLK)]
    return bs
FTAB = 320           # table row: [al_d(4) | al_s(4) | xp(256) | pad(56)]
ROWW = 8 + HID       # 264 useful cols

_cache = {}


def _chunk_rows():
    out = []
    for b0, b1 in _chunks():
        lo = b0 * BLK
        hi = min(b1 * BLK, NSH)
        out.append((lo, hi - lo))
    return out


def _rowmap():
    """node id -> row id in the chunked-AllGather table layout."""
    n = np.arange(N)
    r = n // NSH
    i = n % NSH
    row = np.zeros(N, dtype=np.int64)
    base = 0
    for lo, cnt in _chunk_rows():
        m = (i >= lo) & (i < lo + cnt)
        row[m] = base + r[m] * cnt + (i[m] - lo)
        base += NCORE * cnt
    return row


def _wrap16(idx):
    """[..., NIDX] int -> [..., 128, NIDX//16] int16 wrapped layout:
    element i at [i%16, i//16], replicated across the 8 groups of 16."""
    lead = idx.shape[:-1]
    nidx = idx.shape[-1]
    t = idx.reshape(*lead, nidx // 16, 16)
    t = np.swapaxes(t, -1, -2)  # [..., 16, nidx//16]
    out = np.broadcast_to(t[..., None, :, :], (*lead, 8, 16, nidx // 16))
    return np.ascontiguousarray(out.reshape(*lead, 128, nidx // 16)).astype(np.int16)


def _preprocess(edge_index, batch):
    src = np.concatenate([np.asarray(edge_index[0]), np.arange(N)]).astype(np.int64)
    dst = np.concatenate([np.asarray(edge_index[1]), np.arange(N)]).astype(np.int64)
    order = np.argsort(dst, kind="stable")
    srcs = src[order]
    dsts = dst[order]

    bounds = []
    for c in range(NCORE):
        for b in range(NBLK):
            bounds.append(c * NSH + b * BLK)
    bounds.append(N)
    cuts = np.searchsorted(dsts, np.array(bounds))
    cnts = np.diff(cuts)
    slots = int(np.ceil(cnts.max() / BLK))
    nidx_blk = slots * BLK

    rowmap = _rowmap()
    srcpad = np.zeros((NCORE, NBLK, nidx_blk), dtype=np.int64)
    dloc = np.full((NCORE, NBLK, nidx_blk), -1.0, dtype=np.float32)
    for c in range(NCORE):
        for b in range(NBLK):
            k = c * NBLK + b
            lo, hi = cuts[k], cuts[k + 1]
            n = hi - lo
            srcpad[c, b, :n] = rowmap[srcs[lo:hi]]
            dloc[c, b, :n] = (dsts[lo:hi] - (c * NSH + b * BLK)).astype(np.float32)

    srcw = _wrap16(srcpad.reshape(NCORE, NBLK * nidx_blk))  # [NC,128,NBLK*nidx/16]
    # slot-major dloc for the M01 build: [p, b*SLOTS+s] = dloc[b, s*128+p]
    dl = dloc.reshape(NCORE, NBLK, slots, BLK).transpose(0, 3, 1, 2)
    dl = np.ascontiguousarray(dl.reshape(NCORE, BLK, NBLK * slots))
    # edge-major dloc for the M01T build (partition-broadcast per block)
    dlT = np.ascontiguousarray(dloc.reshape(NCORE, NBLK, nidx_blk))

    bat = np.asarray(batch).astype(np.int64)
    poolv = np.full((NCORE, BLK, NBLK), -1.0, dtype=np.float32)
    for c in range(NCORE):
        for b in range(NBLK):
            lo = c * NSH + b * BLK
            n = min(BLK, c * NSH + NSH - lo)
            poolv[c, :n, b] = bat[lo:lo + n].astype(np.float32)
    return srcw, dl, dlT, poolv, slots


def _amat(att):  # att [H, C] -> [HID, H] block diagonal
    A = np.zeros((H, C, H), dtype=np.float32)
    for h in range(H):
        A[h, :, h] = att[h]
    return A.reshape(HID, H)


def _build(slots):
    import concourse.bacc as bacc
    import concourse.mybir as mybir
    import concourse.tile as tile

    f32 = mybir.dt.float32
    bf16 = mybir.dt.bfloat16
    i16 = mybir.dt.int16
    nidx_blk = slots * BLK
    chunk_rows = _chunk_rows()
    chunks = _chunks()

    nc = bacc.Bacc("TRN2", target_bir_lowering=False, debug=False,
                   enable_asserts=True, num_devices=NCORE)

    xT_d = nc.dram_tensor("xT", [IN, NSH], f32, kind="ExternalInput")
    w0_d = nc.dram_tensor("w0cat", [IN, ROWW], f32, kind="ExternalInput")
    wc_d = nc.dram_tensor("wcat", [128, (NL - 1) * 2 * ROWW], f32, kind="ExternalInput")
    sb_d = nc.dram_tensor("sb", [128, NL * HID], f32, kind="ExternalInput")
    shb_d = nc.dram_tensor("shb", [128, NL * HID], f32, kind="ExternalInput")
    iota_d = nc.dram_tensor("iota", [128, 128], f32, kind="ExternalInput")
    iotac_d = nc.dram_tensor("iotac", [128, 1], f32, kind="ExternalInput")
    iotas_d = nc.dram_tensor("iotas", [128, slots * 128], bf16, kind="ExternalInput")
    ident_d = nc.dram_tensor("ident", [128, 128], f32, kind="ExternalInput")
    srcw_d = nc.dram_tensor("srcw", [128, NBLK * (nidx_blk // 16)], i16, kind="ExternalInput")
    dloc_d = nc.dram_tensor("dloc", [128, NBLK * slots], bf16, kind="ExternalInput")
    dlT_d = nc.dram_tensor("dlT", [NBLK, nidx_blk], bf16, kind="ExternalInput")
    poolv_d = nc.dram_tensor("poolv", [128, NBLK], f32, kind="ExternalInput")

    pool_d = nc.dram_tensor("pool_out", [G, HID], f32, kind="ExternalOutput")

    IW = nidx_blk // 16

    with tile.TileContext(nc) as tc:
        with (
            tc.tile_pool(name="const", bufs=1) as constp,
            tc.tile_pool(name="g1p", bufs=3) as g1p,
            tc.tile_pool(name="work", bufs=2) as workp,
            tc.tile_pool(name="small", bufs=3) as smallp,
            tc.tile_pool(name="psum", bufs=2, space="PSUM") as psump,
            tc.tile_pool(name="psed", bufs=2, space="PSUM") as psedp,
            tc.tile_pool(name="pstr", bufs=1, space="PSUM") as pstrp,
            tc.tile_pool(name="psacc", bufs=1, space="PSUM") as psaccp,
            tc.tile_pool(name="dram", bufs=1, space="DRAM") as dramp,
        ):
            # ---- constants ----
            xT = g1p.tile([IN, NSH], f32, tag="g1", name="xT")
            nc.sync.dma_start(xT[:], xT_d[:])
            w0 = constp.tile([IN, ROWW], f32, tag="w0")
            nc.sync.dma_start(w0[:], w0_d[:])
            wc = constp.tile([128, (NL - 1) * 2 * ROWW], f32, tag="wc")
            nc.sync.dma_start(wc[:], wc_d[:])
            sb = constp.tile([128, NL * HID], f32, tag="sb")
            nc.sync.dma_start(sb[:], sb_d[:])
            shb = constp.tile([128, NL * HID], f32, tag="shb")
            nc.sync.dma_start(shb[:], shb_d[:])
            iota = constp.tile([128, 128], f32, tag="iota")
            nc.sync.dma_start(iota[:], iota_d[:])
            iota_c = constp.tile([128, 1], f32, tag="iotac")
            nc.sync.dma_start(iota_c[:], iotac_d[:])
            iota_s = constp.tile([128, slots, 128], bf16, tag="iotas")
            nc.sync.dma_start(iota_s[:].rearrange("p s d -> p (s d)"), iotas_d[:])
            ident = constp.tile([128, 128], f32, tag="ident")
            nc.sync.dma_start(ident[:], ident_d[:])
            srcw = constp.tile([128, NBLK * IW], i16, tag="srcw")
            nc.sync.dma_start(srcw[:], srcw_d[:])
            dloc = constp.tile([128, NBLK * slots], bf16, tag="dloc")
            nc.sync.dma_start(dloc[:], dloc_d[:])
            poolv = constp.tile([128, NBLK], f32, tag="poolv")
            nc.sync.dma_start(poolv[:], poolv_d[:])

            hT0 = constp.tile([128, NSH], f32, tag="hT0")
            hT1 = constp.tile([128, NSH], f32, tag="hT1")
            alds = [constp.tile([128, NBLK * H], f32, tag=f"ald{l}",
                                name=f"ald{l}")
                    for l in range(NL)]

            pool_ps = psaccp.tile([G, HID], f32, tag="poolps")

            t1s = [tc.tile([N, FTAB], f32, space="DRAM", addr_space="Shared",
                           name=f"t1_{l}")[0] for l in range(NL)]
            agins = [[dramp.tile([cnt, FTAB], f32, tag=f"agin{l}c{k}",
                                 name=f"agin{l}c{k}")
                      for k, (lo, cnt) in enumerate(chunk_rows)]
                     for l in range(NL)]

            def blkw(b):
                return min(BLK, NSH - b * BLK)

            def write_tabrow(lnext, b, ps, w):
                """psum [w, ROWW] -> AllGather input rows + local al_d stash."""
                row = smallp.tile([128, ROWW], f32, tag="tabrow")
                nc.scalar.copy(row[:w, :], ps[:w, :])
                nc.vector.tensor_copy(alds[lnext][:w, b * H:(b + 1) * H],
                                      ps[:w, 0:H])
                for k, (b0, b1) in enumerate(chunks):
                    if b0 <= b < b1:
                        lo = b * BLK - chunk_rows[k][0]
                        dst = agins[lnext][k][lo:lo + w, 0:ROWW]
                        break
                nc.sync.dma_start(dst, row[:w, :])

            def ag_chunk(lnext, k):
                rows = chunk_rows[k][1]
                off = NCORE * sum(cr[1] for cr in chunk_rows[:k])
                out = t1s[lnext][off:off + NCORE * rows, :]
                nc.gpsimd.collective_compute(
                    "AllGather", mybir.AluOpType.bypass,
                    ins=[agins[lnext][k].opt()], outs=[out.opt()],
                    replica_groups=[list(range(NCORE))],
                )

            def maybe_ag(lnext, b):
                for k, (b0, b1) in enumerate(chunks):
                    if b == b1 - 1:
                        ag_chunk(lnext, k)

            # ---- layer-1 table from x ----
            for b in range(NBLK):
                w = blkw(b)
                ps = psump.tile([128, ROWW], f32, tag="xp_ps")
                nc.tensor.matmul(ps[:w, :], xT[:, b * BLK:b * BLK + w], w0[:],
                                 start=True, stop=True)
                write_tabrow(0, b, ps, w)
                maybe_ag(0, b)

            for layer in range(NL):
                t1 = t1s[layer]
                for b in range(NBLK):
                    w = blkw(b)
                    # ---- gather xp+al_s of edge sources ----
                    g1 = g1p.tile([128, slots, FTAB], f32, tag="g1")
                    nc.gpsimd.dma_gather(
                        g1[:], t1[:], srcw[:, b * IW:(b + 1) * IW],
                        num_idxs=nidx_blk, num_idxs_reg=nidx_blk,
                        elem_size=FTAB, single_packet=False)

                    # ---- M01T and al_d -> per-edge broadcast ----
                    dlt = workp.tile([128, nidx_blk], bf16, tag="dlt")
                    nc.sync.dma_start(
                        dlt[:], dlT_d[b:b + 1, :].partition_broadcast(128))
                    m01t = workp.tile([128, nidx_blk], f32, tag="m01t")
                    nc.vector.tensor_scalar(m01t[:], dlt[:], iota_c[:], None,
                                            mybir.AluOpType.is_equal)
                    ed_ps = psedp.tile([128, slots * H], f32, tag="ed_ps")
                    for s in range(slots):
                        nc.tensor.matmul(
                            ed_ps[:, s * H:(s + 1) * H],
                            m01t[:, s * BLK:(s + 1) * BLK],
                            alds[layer][:, b * H:(b + 1) * H],
                            start=True, stop=True)

                    # ---- ex = exp(leakyrelu(al_s + al_d)), scaled messages --
                    gs = workp.tile([128, slots, H + HID], bf16, tag="gs")
                    ext = workp.tile([128, slots, H], f32, tag="ext")
                    nc.vector.tensor_tensor(
                        ext[:], g1[:, :, H:2 * H],
                        ed_ps[:].rearrange("p (s h) -> p s h", h=H),
                        mybir.AluOpType.add)
                    nc.vector.scalar_tensor_tensor(ext[:], ext[:], NEG, ext[:],
                                                   mybir.AluOpType.mult,
                                                   mybir.AluOpType.max)
                    nc.scalar.activation(ext[:], ext[:],
                                         mybir.ActivationFunctionType.Exp)
                    nc.vector.tensor_copy(gs[:, :, 0:H], ext[:])
                    nc.vector.tensor_tensor(
                        gs[:, :, H:H + HID].rearrange("p s (h c) -> p s h c", c=C),
                        g1[:, :, 2 * H:2 * H + HID].rearrange("p s (h c) -> p s h c", c=C),
                        ext[:].broadcast_to([128, slots, H, C]),
                        mybir.AluOpType.mult)
                    m01 = workp.tile([128, slots, 128], bf16, tag="m01")
                    nc.vector.tensor_tensor(
                        m01[:], iota_s[:],
                        dloc[:, b * slots:(b + 1) * slots].broadcast_to([128, slots, 128]),
                        mybir.AluOpType.is_equal)

                    blk_ps = psump.tile([128, H + HID], f32, tag="blk_ps")
                    for s in range(slots):
                        nc.tensor.matmul(blk_ps[:], m01[:, s, :], gs[:, s, :],
                                         start=(s == 0), stop=(s == slots - 1))

                    # ---- normalize, BN + ReLU ----
                    rec = smallp.tile([128, H], f32, tag="rec")
                    nc.vector.tensor_scalar(rec[:w, :], blk_ps[:w, 0:H], 1e-16,
                                            None, mybir.AluOpType.add)
                    nc.vector.reciprocal(rec[:w, :], rec[:w, :])
                    hb = smallp.tile([128, HID], f32, tag="hb")
                    nc.vector.tensor_tensor(
                        hb[:w, :].rearrange("p (h c) -> p h c", c=C),
                        blk_ps[:w, H:H + HID].rearrange("p (h c) -> p h c", c=C),
                        rec[:w, :].broadcast_to([w, H, C]),
                        mybir.AluOpType.mult)
                    nc.vector.tensor_tensor(hb[:w, :], hb[:w, :],
                                            sb[:w, layer * HID:(layer + 1) * HID],
                                            mybir.AluOpType.mult)
                    nc.vector.tensor_tensor(hb[:w, :], hb[:w, :],
                                            shb[:w, layer * HID:(layer + 1) * HID],
                                            mybir.AluOpType.add)
                    nc.scalar.activation(hb[:w, :], hb[:w, :],
                                         mybir.ActivationFunctionType.Relu)

                    if layer < NL - 1:
                        for k in range(2):
                            trp = pstrp.tile([128, 128], f32, tag="trp")
                            nc.tensor.transpose(trp[:], hb[:, k * 128:(k + 1) * 128],
                                                ident[:])
                            hT = hT0 if k == 0 else hT1
                            nc.scalar.copy(hT[:, b * BLK:b * BLK + w], trp[:, :w])
                        xp_ps = psump.tile([128, ROWW], f32, tag="xp_ps")
                        for k in range(2):
                            hT = hT0 if k == 0 else hT1
                            nc.tensor.matmul(
                                xp_ps[:w, :], hT[:, b * BLK:b * BLK + w],
                                wc[:, (layer * 2 + k) * ROWW:(layer * 2 + k + 1) * ROWW],
                                start=(k == 0), stop=(k == 1))
                        write_tabrow(layer + 1, b, xp_ps, w)
                        maybe_ag(layer + 1, b)
                    else:
                        mp = smallp.tile([128, G], f32, tag="mpool")
                        nc.vector.tensor_scalar(
                            mp[:], iota[:, 0:G], poolv[:, b:b + 1], None,
                            mybir.AluOpType.is_equal)
                        nc.tensor.matmul(pool_ps[:], mp[:], hb[:],
                                         start=(b == 0), stop=(b == NBLK - 1),
                                         skip_group_check=True)

            pout = smallp.tile([G, HID], f32, tag="pout")
            nc.scalar.copy(pout[:], pool_ps[:])
            nc.sync.dma_start(pool_d[:], pout[:])

    nc.compile()
    return nc


def _host_inputs(inputs, srcw, dl, dlT, poolv, slots):
    x = np.asarray(inputs["x"], dtype=np.float32)
    W0 = np.asarray(inputs["W0"], dtype=np.float32)
    W_rest = np.asarray(inputs["W_rest"], dtype=np.float32)
    att_src = np.asarray(inputs["att_src"], dtype=np.float32)
    att_dst = np.asarray(inputs["att_dst"], dtype=np.float32)
    bias_conv = np.asarray(inputs["bias_conv"], dtype=np.float32)
    bn_gamma = np.asarray(inputs["bn_gamma"], dtype=np.float32)
    bn_beta = np.asarray(inputs["bn_beta"], dtype=np.float32)
    bn_mean = np.asarray(inputs["bn_mean"], dtype=np.float32)
    bn_var = np.asarray(inputs["bn_var"], dtype=np.float32)

    wcats = []
    for layer in range(NL):
        Wl = W0 if layer == 0 else W_rest[layer - 1]
        Ad = _amat(att_dst[layer])
        As = _amat(att_src[layer])
        wcat = np.concatenate([Wl @ Ad, Wl @ As, Wl], axis=1)  # [in, 264]
        wcats.append(wcat.astype(np.float32))
    w0cat = wcats[0]
    wcat = np.concatenate(
        [w.reshape(2, 128, ROWW)[k] for w in wcats[1:] for k in range(2)], axis=1)
    wcat = np.ascontiguousarray(wcat)

    s = bn_gamma / np.sqrt(bn_var + BN_EPS)
    shift = (bias_conv - bn_mean) * s + bn_beta
    sb = np.ascontiguousarray(np.broadcast_to(s.reshape(-1), (128, NL * HID)))
    shb = np.ascontiguousarray(np.broadcast_to(shift.reshape(-1), (128, NL * HID)))

    iota = np.ascontiguousarray(
        np.broadcast_to(np.arange(128, dtype=np.float32), (128, 128)))
    import ml_dtypes
    bf = ml_dtypes.bfloat16
    iotac = np.arange(128, dtype=np.float32).reshape(128, 1)
    iotas = np.ascontiguousarray(np.broadcast_to(
        np.tile(np.arange(128, dtype=bf), slots), (128, slots * 128)))
    ident = np.eye(128, dtype=np.float32)

    in_maps = []
    for c in range(NCORE):
        xs = x[c * NSH:(c + 1) * NSH, :]
        in_maps.append(dict(
            xT=np.ascontiguousarray(xs.T),
            w0cat=w0cat, wcat=wcat, sb=sb, shb=shb,
            iota=iota, iotac=iotac, iotas=iotas, ident=ident,
            srcw=srcw[c], dloc=dl[c].astype(bf), dlT=dlT[c].astype(bf),
            poolv=poolv[c],
        ))
    return in_maps


def _postprocess(partials, inputs):
    pooled = np.sum(np.stack(partials), axis=0)
    lg = np.asarray(inputs["lbn_gamma"], dtype=np.float32)
    lb = np.asarray(inputs["lbn_beta"], dtype=np.float32)
    lm = np.asarray(inputs["lbn_mean"], dtype=np.float32)
    lv = np.asarray(inputs["lbn_var"], dtype=np.float32)
    fw = np.asarray(inputs["fc_W"], dtype=np.float32)
    fb = np.asarray(inputs["fc_b"], dtype=np.float32)
    pooled = (pooled - lm) / np.sqrt(lv + BN_EPS) * lg + lb
    return (pooled @ fw + fb).astype(np.float32)


def _get_program(slots):
    key = ("prog", slots)
    if key not in _cache:
        _cache[key] = _build(slots)
    return _cache[key]


def run(inputs, trace=False, trace_kwargs=None):
    from concourse.bass_utils import run_bass_kernel_spmd
    srcw, dl, dlT, poolv, slots = _preprocess(inputs["edge_index"], inputs["batch"])
    nc = _get_program(slots)
    in_maps = _host_inputs(inputs, srcw, dl, dlT, poolv, slots)
    res = run_bass_kernel_spmd(nc, in_maps, list(range(NCORE)),
                               trace=trace, **(trace_kwargs or {}))
    partials = [res.results[c]["pool_out"] for c in range(NCORE)]
    return _postprocess(partials, inputs), res


def kernel(**inputs) -> np.ndarray:
    out, _ = run(inputs)
    return out


# revision 14
# speedup vs baseline: 1.8597x; 1.0001x over previous
"""3-layer GAT + global pool on 8 Trainium2 NeuronCores (Bass/Tile SPMD).

Strategy: shard edges by destination node across the 8 cores (2500 dst
nodes each). Per layer, each core computes the node table
[al_d | al_s | xp] for its node shard; two AllGather collectives (split
in halves so the first overlaps trailing compute) replicate the full
table to Shared DRAM on every core. All per-edge work is core-local:
one dma_gather by src per 128-dst-node block, attention coefficients
exp(leakyrelu(al_s[src]+al_d[dst])) with the dst-side broadcast done by
a transposed one-hot matmul (no per-edge gather), and the weighted
scatter-add done as one-hot matmuls accumulating [denom | sum ex*xp] in
PSUM. Final pooled [64, HID] partials are summed on the host, which
also applies the (tiny) final BatchNorm + FC.
"""
import numpy as np

# ---- model constants (must match the reference) ----
N = 20000
E = 320000
G = 64
H = 4
C = 64
HID = H * C          # 256
IN = 128
LAT = 64
NEG = 0.2
BN_EPS = 1e-5
NL = 3

NCORE = 8
NSH = N // NCORE     # 2500 dst nodes per core
BLK = 128
NBLK = (NSH + BLK - 1) // BLK   # 20 (last block 68 nodes)
def _chunks():
    "AllGather chunks as (block_lo, block_hi) ranges."
    if NBLK >= 20:
        bs = [(0, 8), (8, 16), (16, 19), (19, 20)]
    elif NBLK >= 2:
        bs = [(0, NBLK - 1), (NBLK - 1, NBLK)]
    else:
        bs = [(0, NBLK)]
    return bs
FTAB = 320           # table row: [al_d(4) | al_s(4) | xp(256) | pad(56)]
ROWW = 8 + HID       # 264 useful cols

_cache = {}


def _chunk_rows():
    out = []
    for b0, b1 in _chunks():
        lo = b0 * BLK
        hi = min(b1 * BLK, NSH)
        out.append((lo, hi - lo))
    return out


def _rowmap():
    """node id -> row id in the chunked-AllGather table layout."""
    n = np.arange(N)
    r = n // NSH
    i = n % NSH
    row = np.zeros(N, dtype=np.int64)
    base = 0
    for lo, cnt in _chunk_rows():
        m = (i >= lo) & (i < lo + cnt)
        row[m] = base + r[m] * cnt + (i[m] - lo)
        base += NCORE * cnt
    return row


def _wrap16(idx):
    """[..., NIDX] int -> [..., 128, NIDX//16] int16 wrapped layout:
    element i at [i%16, i//16], replicated across the 8 groups of 16."""
    lead = idx.shape[:-1]
    nidx = idx.shape[-1]
    t = idx.reshape(*lead, nidx // 16, 16)
    t = np.swapaxes(t, -1, -2)  # [..., 16, nidx//16]
    out = np.broadcast_to(t[..., None, :, :], (*lead, 8, 16, nidx // 16))
    return np.ascontiguousarray(out.reshape(*lead, 128, nidx // 16)).astype(np.int16)


def _preprocess(edge_index, batch):
    src = np.concatenate([np.asarray(edge_index[0]), np.arange(N)]).astype(np.int64)
    dst = np.concatenate([np.asarray(edge_index[1]), np.arange(N)]).astype(np.int64)
    order = np.argsort(dst, kind="stable")
    srcs = src[order]
    dsts = dst[order]

    bounds = []
    for c in range(NCORE):
        for b in range(NBLK):
            bounds.append(c * NSH + b * BLK)
    bounds.append(N)
    cuts = np.searchsorted(dsts, np.array(bounds))
    cnts = np.diff(cuts)
    slots = int(np.ceil(cnts.max() / BLK))
    nidx_blk = slots * BLK

    rowmap = _rowmap()
    srcpad = np.zeros((NCORE, NBLK, nidx_blk), dtype=np.int64)
    dloc = np.full((NCORE, NBLK, nidx_blk), -1.0, dtype=np.float32)
    for c in range(NCORE):
        for b in range(NBLK):
            k = c * NBLK + b
            lo, hi = cuts[k], cuts[k + 1]
            n = hi - lo
            srcpad[c, b, :n] = rowmap[srcs[lo:hi]]
            dloc[c, b, :n] = (dsts[lo:hi] - (c * NSH + b * BLK)).astype(np.float32)

    srcw = _wrap16(srcpad.reshape(NCORE, NBLK * nidx_blk))  # [NC,128,NBLK*nidx/16]
    # slot-major dloc for the M01 build: [p, b*SLOTS+s] = dloc[b, s*128+p]
    dl = dloc.reshape(NCORE, NBLK, slots, BLK).transpose(0, 3, 1, 2)
    dl = np.ascontiguousarray(dl.reshape(NCORE, BLK, NBLK * slots))
    # edge-major dloc for the M01T build (partition-broadcast per block)
    dlT = np.ascontiguousarray(dloc.reshape(NCORE, NBLK, nidx_blk))

    bat = np.asarray(batch).astype(np.int64)
    poolv = np.full((NCORE, BLK, NBLK), -1.0, dtype=np.float32)
    for c in range(NCORE):
        for b in range(NBLK):
            lo = c * NSH + b * BLK
            n = min(BLK, c * NSH + NSH - lo)
            poolv[c, :n, b] = bat[lo:lo + n].astype(np.float32)
    return srcw, dl, dlT, poolv, slots


def _amat(att):  # att [H, C] -> [HID, H] block diagonal
    A = np.zeros((H, C, H), dtype=np.float32)
    for h in range(H):
        A[h, :, h] = att[h]
    return A.reshape(HID, H)


def _build(slots):
    import concourse.bacc as bacc
    import concourse.mybir as mybir
    import concourse.tile as tile

    f32 = mybir.dt.float32
    bf16 = mybir.dt.bfloat16
    i16 = mybir.dt.int16
    nidx_blk = slots * BLK
    chunk_rows = _chunk_rows()
    chunks = _chunks()

    nc = bacc.Bacc("TRN2", target_bir_lowering=False, debug=False,
                   enable_asserts=True, num_devices=NCORE)

    xT_d = nc.dram_tensor("xT", [IN, NSH], f32, kind="ExternalInput")
    w0_d = nc.dram_tensor("w0cat", [IN, ROWW], f32, kind="ExternalInput")
    wc_d = nc.dram_tensor("wcat", [128, (NL - 1) * 2 * ROWW], f32, kind="ExternalInput")
    sb_d = nc.dram_tensor("sb", [128, NL * HID], f32, kind="ExternalInput")
    shb_d = nc.dram_tensor("shb", [128, NL * HID], f32, kind="ExternalInput")
    iota_d = nc.dram_tensor("iota", [128, 128], f32, kind="ExternalInput")
    iotac_d = nc.dram_tensor("iotac", [128, 1], f32, kind="ExternalInput")
    iotas_d = nc.dram_tensor("iotas", [128, slots * 128], bf16, kind="ExternalInput")
    ident_d = nc.dram_tensor("ident", [128, 128], f32, kind="ExternalInput")
    srcw_d = nc.dram_tensor("srcw", [128, NBLK * (nidx_blk // 16)], i16, kind="ExternalInput")
    dloc_d = nc.dram_tensor("dloc", [128, NBLK * slots], bf16, kind="ExternalInput")
    dlT_d = nc.dram_tensor("dlT", [NBLK, nidx_blk], bf16, kind="ExternalInput")
    poolv_d = nc.dram_tensor("poolv", [128, NBLK], f32, kind="ExternalInput")

    pool_d = nc.dram_tensor("pool_out", [G, HID], f32, kind="ExternalOutput")

    IW = nidx_blk // 16

    with tile.TileContext(nc) as tc:
        with (
            tc.tile_pool(name="const", bufs=1) as constp,
            tc.tile_pool(name="g1p", bufs=3) as g1p,
            tc.tile_pool(name="work", bufs=2) as workp,
            tc.tile_pool(name="small", bufs=3) as smallp,
            tc.tile_pool(name="psum", bufs=2, space="PSUM") as psump,
            tc.tile_pool(name="psed", bufs=2, space="PSUM") as psedp,
            tc.tile_pool(name="pstr", bufs=1, space="PSUM") as pstrp,
            tc.tile_pool(name="psacc", bufs=1, space="PSUM") as psaccp,
            tc.tile_pool(name="dram", bufs=1, space="DRAM") as dramp,
        ):
            # ---- constants ----
            xT = g1p.tile([IN, NSH], f32, tag="g1", name="xT")
            nc.sync.dma_start(xT[:], xT_d[:])
            w0 = constp.tile([IN, ROWW], f32, tag="w0")
            nc.sync.dma_start(w0[:], w0_d[:])
            wc = constp.tile([128, (NL - 1) * 2 * ROWW], f32, tag="wc")
            nc.sync.dma_start(wc[:], wc_d[:])
            sb = constp.tile([128, NL * HID], f32, tag="sb")
            nc.sync.dma_start(sb[:], sb_d[:])
            shb = constp.tile([128, NL * HID], f32, tag="shb")
            nc.sync.dma_start(shb[:], shb_d[:])
            iota = constp.tile([128, 128], f32, tag="iota")
            nc.sync.dma_start(iota[:], iota_d[:])
            iota_c = constp.tile([128, 1], f32, tag="iotac")
            nc.sync.dma_start(iota_c[:], iotac_d[:])
            iota_s = constp.tile([128, slots, 128], bf16, tag="iotas")
            nc.sync.dma_start(iota_s[:].rearrange("p s d -> p (s d)"), iotas_d[:])
            ident = constp.tile([128, 128], f32, tag="ident")
            nc.sync.dma_start(ident[:], ident_d[:])
            srcw = constp.tile([128, NBLK * IW], i16, tag="srcw")
            nc.sync.dma_start(srcw[:], srcw_d[:])
            dloc = constp.tile([128, NBLK * slots], bf16, tag="dloc")
            nc.sync.dma_start(dloc[:], dloc_d[:])
            poolv = constp.tile([128, NBLK], f32, tag="poolv")
            nc.sync.dma_start(poolv[:], poolv_d[:])

            hT0 = constp.tile([128, NSH], f32, tag="hT0")
            hT1 = constp.tile([128, NSH], f32, tag="hT1")
            alds = [constp.tile([128, NBLK * H], f32, tag=f"ald{l}",
                                name=f"ald{l}")
                    for l in range(NL)]

            pool_ps = psaccp.tile([G, HID], f32, tag="poolps")

            t1s = [tc.tile([N, FTAB], f32, space="DRAM", addr_space="Shared",
                           name=f"t1_{l}")[0] for l in range(NL)]
            agins = [[dramp.tile([cnt, FTAB], f32, tag=f"agin{l}c{k}",
                                 name=f"agin{l}c{k}")
                      for k, (lo, cnt) in enumerate(chunk_rows)]
                     for l in range(NL)]

            def blkw(b):
                return min(BLK, NSH - b * BLK)

            def write_tabrow(lnext, b, ps, w):
                """psum [w, ROWW] -> AllGather input rows + local al_d stash."""
                row = smallp.tile([128, ROWW], f32, tag="tabrow")
                nc.scalar.copy(row[:w, :], ps[:w, :])
                nc.vector.tensor_copy(alds[lnext][:w, b * H:(b + 1) * H],
                                      ps[:w, 0:H])
                for k, (b0, b1) in enumerate(chunks):
                    if b0 <= b < b1:
                        lo = b * BLK - chunk_rows[k][0]
                        dst = agins[lnext][k][lo:lo + w, 0:ROWW]
                        break
                nc.sync.dma_start(dst, row[:w, :])

            def ag_chunk(lnext, k):
                rows = chunk_rows[k][1]
                off = NCORE * sum(cr[1] for cr in chunk_rows[:k])
                out = t1s[lnext][off:off + NCORE * rows, :]
                nc.gpsimd.collective_compute(
                    "AllGather", mybir.AluOpType.bypass,
                    ins=[agins[lnext][k].opt()], outs=[out.opt()],
                    replica_groups=[list(range(NCORE))],
                )

            def maybe_ag(lnext, b):
                for k, (b0, b1) in enumerate(chunks):
                    if b == b1 - 1:
                        ag_chunk(lnext, k)

            # ---- layer-1 table from x ----
            for b in range(NBLK):
                w = blkw(b)
                ps = psump.tile([128, ROWW], f32, tag="xp_ps")
                nc.tensor.matmul(ps[:w, :], xT[:, b * BLK:b * BLK + w], w0[:],
                                 start=True, stop=True)
                write_tabrow(0, b, ps, w)
                maybe_ag(0, b)

            for layer in range(NL):
                t1 = t1s[layer]
                LAG = 5
                fire_at = {}
                if layer < NL - 1:
                    for k, (b0, b1) in enumerate(chunks):
                        f = b1 - 1 + LAG
                        if f < NBLK:
                            fire_at.setdefault(f, []).append(k)
                for b in range(NBLK):
                    w = blkw(b)
                    for k in fire_at.get(b, []):
                        ag_chunk(layer + 1, k)
                    # ---- gather xp+al_s of edge sources ----
                    g1 = g1p.tile([128, slots, FTAB], f32, tag="g1")
                    nc.gpsimd.dma_gather(
                        g1[:], t1[:], srcw[:, b * IW:(b + 1) * IW],
                        num_idxs=nidx_blk, num_idxs_reg=nidx_blk,
                        elem_size=FTAB, single_packet=False)

                    # ---- M01T and al_d -> per-edge broadcast ----
                    dlt = workp.tile([128, nidx_blk], bf16, tag="dlt")
                    nc.sync.dma_start(
                        dlt[:], dlT_d[b:b + 1, :].partition_broadcast(128))
                    m01t = workp.tile([128, nidx_blk], f32, tag="m01t")
                    nc.vector.tensor_scalar(m01t[:], dlt[:], iota_c[:], None,
                                            mybir.AluOpType.is_equal)
                    ed_ps = psedp.tile([128, slots * H], f32, tag="ed_ps")
                    for s in range(slots):
                        nc.tensor.matmul(
                            ed_ps[:, s * H:(s + 1) * H],
                            m01t[:, s * BLK:(s + 1) * BLK],
                            alds[layer][:, b * H:(b + 1) * H],
                            start=True, stop=True)

                    # ---- ex = exp(leakyrelu(al_s + al_d)), scaled messages --
                    gs = workp.tile([128, slots, H + HID], bf16, tag="gs")
                    ext = workp.tile([128, slots, H], f32, tag="ext")
                    nc.vector.tensor_tensor(
                        ext[:], g1[:, :, H:2 * H],
                        ed_ps[:].rearrange("p (s h) -> p s h", h=H),
                        mybir.AluOpType.add)
                    nc.vector.scalar_tensor_tensor(ext[:], ext[:], NEG, ext[:],
                                                   mybir.AluOpType.mult,
                                                   mybir.AluOpType.max)
                    nc.scalar.activation(ext[:], ext[:],
                                         mybir.ActivationFunctionType.Exp)
                    nc.vector.tensor_copy(gs[:, :, 0:H], ext[:])
                    nc.vector.tensor_tensor(
                        gs[:, :, H:H + HID].rearrange("p s (h c) -> p s h c", c=C),
                        g1[:, :, 2 * H:2 * H + HID].rearrange("p s (h c) -> p s h c", c=C),
                        ext[:].broadcast_to([128, slots, H, C]),
                        mybir.AluOpType.mult)
                    m01 = workp.tile([128, slots, 128], bf16, tag="m01")
                    nc.vector.tensor_tensor(
                        m01[:], iota_s[:],
                        dloc[:, b * slots:(b + 1) * slots].broadcast_to([128, slots, 128]),
                        mybir.AluOpType.is_equal)

                    blk_ps = psump.tile([128, H + HID], f32, tag="blk_ps")
                    for s in range(slots):
                        nc.tensor.matmul(blk_ps[:], m01[:, s, :], gs[:, s, :],
                                         start=(s == 0), stop=(s == slots - 1))

                    # ---- normalize, BN + ReLU ----
                    rec = smallp.tile([128, H], f32, tag="rec")
                    nc.vector.tensor_scalar(rec[:w, :], blk_ps[:w, 0:H], 1e-16,
                                            None, mybir.AluOpType.add)
                    nc.vector.reciprocal(rec[:w, :], rec[:w, :])
                    hb = smallp.tile([128, HID], f32, tag="hb")
                    nc.vector.tensor_tensor(
                        hb[:w, :].rearrange("p (h c) -> p h c", c=C),
                        blk_ps[:w, H:H + HID].rearrange("p (h c) -> p h c", c=C),
                        rec[:w, :].broadcast_to([w, H, C]),
                        mybir.AluOpType.mult)
                    nc.vector.tensor_tensor(hb[:w, :], hb[:w, :],
                                            sb[:w, layer * HID:(layer + 1) * HID],
                                            mybir.AluOpType.mult)
                    nc.vector.tensor_tensor(hb[:w, :], hb[:w, :],
                                            shb[:w, layer * HID:(layer + 1) * HID],
                                            mybir.AluOpType.add)
                    nc.scalar.activation(hb[:w, :], hb[:w, :],
                                         mybir.ActivationFunctionType.Relu)

                    if layer < NL - 1:
                        for k in range(2):
                            trp = pstrp.tile([128, 128], f32, tag="trp")
                            nc.tensor.transpose(trp[:], hb[:, k * 128:(k + 1) * 128],
                                                ident[:])
                            hT = hT0 if k == 0 else hT1
                            nc.scalar.copy(hT[:, b * BLK:b * BLK + w], trp[:, :w])
                        xp_ps = psump.tile([128, ROWW], f32, tag="xp_ps")
                        for k in range(2):
                            hT = hT0 if k == 0 else hT1
                            nc.tensor.matmul(
                                xp_ps[:w, :], hT[:, b * BLK:b * BLK + w],
                                wc[:, (layer * 2 + k) * ROWW:(layer * 2 + k + 1) * ROWW],
                                start=(k == 0), stop=(k == 1))
                        write_tabrow(layer + 1, b, xp_ps, w)
                    else:
                        mp = smallp.tile([128, G], f32, tag="mpool")
                        nc.vector.tensor_scalar(
                            mp[:], iota[:, 0:G], poolv[:, b:b + 1], None,
                            mybir.AluOpType.is_equal)
                        nc.tensor.matmul(pool_ps[:], mp[:], hb[:],
                                         start=(b == 0), stop=(b == NBLK - 1),
                                         skip_group_check=True)
                if layer < NL - 1:
                    for k, (b0, b1) in enumerate(chunks):
                        if b1 - 1 + LAG >= NBLK:
                            ag_chunk(layer + 1, k)

            pout = smallp.tile([G, HID], f32, tag="pout")
            nc.scalar.copy(pout[:], pool_ps[:])
            nc.sync.dma_start(pool_d[:], pout[:])

    nc.compile()
    return nc


def _host_inputs(inputs, srcw, dl, dlT, poolv, slots):
    x = np.asarray(inputs["x"], dtype=np.float32)
    W0 = np.asarray(inputs["W0"], dtype=np.float32)
    W_rest = np.asarray(inputs["W_rest"], dtype=np.float32)
    att_src = np.asarray(inputs["att_src"], dtype=np.float32)
    att_dst = np.asarray(inputs["att_dst"], dtype=np.float32)
    bias_conv = np.asarray(inputs["bias_conv"], dtype=np.float32)
    bn_gamma = np.asarray(inputs["bn_gamma"], dtype=np.float32)
    bn_beta = np.asarray(inputs["bn_beta"], dtype=np.float32)
    bn_mean = np.asarray(inputs["bn_mean"], dtype=np.float32)
    bn_var = np.asarray(inputs["bn_var"], dtype=np.float32)

    wcats = []
    for layer in range(NL):
        Wl = W0 if layer == 0 else W_rest[layer - 1]
        Ad = _amat(att_dst[layer])
        As = _amat(att_src[layer])
        wcat = np.concatenate([Wl @ Ad, Wl @ As, Wl], axis=1)  # [in, 264]
        wcats.append(wcat.astype(np.float32))
    w0cat = wcats[0]
    wcat = np.concatenate(
        [w.reshape(2, 128, ROWW)[k] for w in wcats[1:] for k in range(2)], axis=1)
    wcat = np.ascontiguousarray(wcat)

    s = bn_gamma / np.sqrt(bn_var + BN_EPS)
    shift = (bias_conv - bn_mean) * s + bn_beta
    sb = np.ascontiguousarray(np.broadcast_to(s.reshape(-1), (128, NL * HID)))
    shb = np.ascontiguousarray(np.broadcast_to(shift.reshape(-1), (128, NL * HID)))

    iota = np.ascontiguousarray(
        np.broadcast_to(np.arange(128, dtype=np.float32), (128, 128)))
    import ml_dtypes
    bf = ml_dtypes.bfloat16
    iotac = np.arange(128, dtype=np.float32).reshape(128, 1)
    iotas = np.ascontiguousarray(np.broadcast_to(
        np.tile(np.arange(128, dtype=bf), slots), (128, slots * 128)))
    ident = np.eye(128, dtype=np.float32)

    in_maps = []
    for c in range(NCORE):
        xs = x[c * NSH:(c + 1) * NSH, :]
        in_maps.append(dict(
            xT=np.ascontiguousarray(xs.T),
            w0cat=w0cat, wcat=wcat, sb=sb, shb=shb,
            iota=iota, iotac=iotac, iotas=iotas, ident=ident,
            srcw=srcw[c], dloc=dl[c].astype(bf), dlT=dlT[c].astype(bf),
            poolv=poolv[c],
        ))
    return in_maps


def _postprocess(partials, inputs):
    pooled = np.sum(np.stack(partials), axis=0)
    lg = np.asarray(inputs["lbn_gamma"], dtype=np.float32)
    lb = np.asarray(inputs["lbn_beta"], dtype=np.float32)
    lm = np.asarray(inputs["lbn_mean"], dtype=np.float32)
    lv = np.asarray(inputs["lbn_var"], dtype=np.float32)
    fw = np.asarray(inputs["fc_W"], dtype=np.float32)
    fb = np.asarray(inputs["fc_b"], dtype=np.float32)
    pooled = (pooled - lm) / np.sqrt(lv + BN_EPS) * lg + lb
    return (pooled @ fw + fb).astype(np.float32)


def _get_program(slots):
    key = ("prog", slots)
    if key not in _cache:
        _cache[key] = _build(slots)
    return _cache[key]


def run(inputs, trace=False, trace_kwargs=None):
    from concourse.bass_utils import run_bass_kernel_spmd
    srcw, dl, dlT, poolv, slots = _preprocess(inputs["edge_index"], inputs["batch"])
    nc = _get_program(slots)
    in_maps = _host_inputs(inputs, srcw, dl, dlT, poolv, slots)
    res = run_bass_kernel_spmd(nc, in_maps, list(range(NCORE)),
                               trace=trace, **(trace_kwargs or {}))
    partials = [res.results[c]["pool_out"] for c in range(NCORE)]
    return _postprocess(partials, inputs), res


def kernel(**inputs) -> np.ndarray:
    out, _ = run(inputs)
    return out
